# revision 1
# baseline (speedup 1.0000x reference)
"""Trainium2 Bass kernel for a hard-triplet margin-ranking loss.

Sharding: data-parallel over anchor rows. 8 cores x 512 rows each. Rows in
the first half of the batch mine over columns [2048:4096], rows in the second
half over [0:2048], so each core needs only its 512x2048 slice of the
distance matrix. Per core:

  1. Load features in five [128, 4x256] group tiles (separate DMAs so
     compute pipelines with the loads).
  2. Row norms via ACT Square+accum; inv = 1/(sqrt(sq)+eps) (DVE reciprocal).
     Anchor rows are scaled by -0.25*inv, opposite rows by inv, so the PE
     matmul yields pm = -0.25*<xn_i, xn_j> in [-0.25, 0.25] and
     dist^2 = 2 + 8*pm (||xn||^2 deviates from 1.0 by <= 2e-7, far below
     the fp32 noise of the reference).
  3. Normalize on GPSIMD (tensor_scalar), transpose via PE identity matmuls,
     evacuate PSUM per group with one ACT copy into per-group xoT tiles so
     each matmul column chunk can start as soon as its group lands.
  4. pm = xbT.T @ xoT on PE (fp32, K=256 accumulated in PSUM).
  5. Same-class mask fused with the PSUM read: one DVE scalar_tensor_tensor
     w = (t_o == t_b) + pm; matched columns land in [0.75, 1.25], unmatched
     in [-0.25, 0.25].  Row max -> hardest positive, row min -> hardest
     negative (squared space; sqrt only on the reduced values).
  6. dist_ap = sqrt(relu(8*mx - 6)) (exact 0 when a row has no positives),
     dist_an = sqrt(max(8*mn + 2, eps)) or 1.0 when a row has no negatives
     (then 8*mn + 2 >= 8 > 6 >= any real dist^2),
     row loss = relu(dist_ap - dist_an + margin); ones-matmul row-sum.
  7. Host sums the 8 per-core partial sums / 4096.
"""

import numpy as np

N, D = 4096, 256
HALF = N // 2
NCORES = 8
RPC = N // NCORES  # 512 anchor rows per core
RT = RPC // 128    # 4 anchor row tiles
OT = HALF // 128   # 16 opposite-half tiles
NT = RT + OT       # 20 input tiles
NG = NT // 4       # 5 groups of 4 tiles
MARGIN = 0.3
EPS = 1e-6
S = 0.125          # anchor pre-scale: pm = -2*S*dot = -0.25*dot

_CACHE = {}


def _build():
    from contextlib import ExitStack

    import concourse.bacc as bacc
    import concourse.bass as bass
    import concourse.tile as tile
    from concourse import masks, mybir

    f32 = mybir.dt.float32
    Alu = mybir.AluOpType
    Act = mybir.ActivationFunctionType
    AxX = mybir.AxisListType.X
    ts = bass.ts

    nc = bacc.Bacc(
        "TRN2",
        target_bir_lowering=False,
        debug=False,
        enable_asserts=True,
        num_devices=NCORES,
    )
    xb = nc.dram_tensor("xb", [128, RT * D], f32, kind="ExternalInput").ap()
    xo = nc.dram_tensor("xo", [128, OT * D], f32, kind="ExternalInput").ap()
    tb = nc.dram_tensor("tb", [128, RT], f32, kind="ExternalInput").ap()
    to = nc.dram_tensor("to", [1, HALF], f32, kind="ExternalInput").ap()
    out = nc.dram_tensor("out", [1, 1], f32, kind="ExternalOutput").ap()

    with tile.TileContext(nc) as tc, ExitStack() as ctx:
        const = ctx.enter_context(tc.tile_pool(name="const", bufs=1))
        xin = ctx.enter_context(tc.tile_pool(name="xin", bufs=1))
        xt = ctx.enter_context(tc.tile_pool(name="xt", bufs=1))
        stat = ctx.enter_context(tc.tile_pool(name="stat", bufs=1))
        scr = ctx.enter_context(tc.tile_pool(name="scr", bufs=3))
        wide = ctx.enter_context(tc.tile_pool(name="wide", bufs=2))
        psum = ctx.enter_context(tc.tile_pool(name="psum", bufs=2, space="PSUM"))

        ident = const.tile([128, 128], f32, tag="ident")
        masks.make_identity(nc, ident[:])
        ones = const.tile([128, 1], f32, tag="ones")
        nc.vector.memset(ones[:], 1.0)

        # Targets: opposite-half row broadcast to all partitions; per-row
        # targets as one [128, RT] per-partition scalar bank.
        to_row = const.tile([1, HALF], f32, tag="to_row")
        nc.sync.dma_start(to_row[:], to[:])
        tob = const.tile([128, HALF], f32, tag="tob")
        nc.gpsimd.partition_broadcast(tob[:], to_row[:])
        tbt = const.tile([128, RT], f32, tag="tbt")
        nc.sync.dma_start(tbt[:], tb[:])

        # Feature tiles in 5 groups of 4: group 0 = anchors, 1..4 = opposite.
        xg = []
        for g in range(NG):
            gt = xin.tile([128, 4 * D], f32, tag=f"xg{g}")
            if g == 0:
                nc.sync.dma_start(gt[:], xb[:])
            else:
                nc.sync.dma_start(gt[:], xo[:, (g - 1) * 4 * D : g * 4 * D])
            xg.append(gt)

        # Row norms: sq[p, t] = sum_d x[p+128t, d]^2, one ACT op per tile.
        sq = stat.tile([128, NT], f32, tag="sq")
        for t in range(NT):
            s = scr.tile([128, D], f32, tag="sq_scratch")
            nc.scalar.activation(
                s[:], xg[t // 4][:, ts(t % 4, D)], Act.Square,
                accum_out=sq[:, t : t + 1],
            )
        nrm = stat.tile([128, NT], f32, tag="nrm")
        nc.scalar.activation(nrm[:], sq[:], Act.Sqrt)
        nrme = stat.tile([128, NT], f32, tag="nrme")
        nc.vector.tensor_scalar_add(nrme[:], nrm[:], EPS)
        inv = stat.tile([128, NT], f32, tag="inv")
        nc.vector.reciprocal(inv[:], nrme[:])
        inv2 = stat.tile([128, RT], f32, tag="inv2")
        nc.vector.tensor_scalar_mul(inv2[:], inv[:, 0:RT], -2.0 * S)

        # Normalize (GPSIMD) + PE-transpose per group; evacuate with one ACT
        # copy per group.  Group g tile layout: [128 dims(c), 512 rows] at
        # columns [c*512, (c+1)*512).
        xT = []
        for g in range(NG):
            gt = xt.tile([128, 1024], f32, tag=f"xT{g}")
            pt = psum.tile([128, 1024], f32, tag="ps")
            for i in range(4):
                t = g * 4 + i
                xn = scr.tile([128, D], f32, tag="xn")
                sc = inv2[:, t : t + 1] if t < RT else inv[:, t : t + 1]
                nc.vector.tensor_scalar_mul(xn[:], xg[g][:, ts(i, D)], sc)
                for c in range(2):
                    nc.tensor.transpose(
                        pt[:, ts(c * 4 + i, 128)], xn[:, ts(c, 128)], ident[:]
                    )
            nc.scalar.copy(gt[:], pt[:])
            xT.append(gt)

        # Main matmul + fused mask + row max/min, per anchor row tile.
        mx = stat.tile([128, RT], f32, tag="mx")
        mn = stat.tile([128, RT], f32, tag="mn")
        for r in range(RT):
            pm = psum.tile([128, 2048], f32, tag="ps")
            for n in range(4):
                for c in range(2):
                    nc.tensor.matmul(
                        pm[:, ts(n, 512)],
                        lhsT=xT[0][:, c * RPC + r * 128 : c * RPC + (r + 1) * 128],
                        rhs=xT[1 + n][:, ts(c, 512)],
                        start=(c == 0),
                        stop=(c == 1),
                    )
            w = wide.tile([128, HALF], mybir.dt.float16, tag="w")
            nc.vector.scalar_tensor_tensor(
                w[:], tob[:], tbt[:, r : r + 1], pm[:],
                op0=Alu.is_equal, op1=Alu.add,
            )
            nc.vector.tensor_reduce(mx[:, r : r + 1], w[:], axis=AxX, op=Alu.max)
            nc.vector.tensor_reduce(mn[:, r : r + 1], w[:], axis=AxX, op=Alu.min)

        # Epilogue on [128, RT]:
        # dist_ap^2 = relu(8*mx - 6); exact 0 when row has no positives.
        u1 = stat.tile([128, RT], f32, tag="u1")
        nc.vector.tensor_scalar(u1[:], mx[:], 8.0, -6.0, op0=Alu.mult, op1=Alu.add)
        u = stat.tile([128, RT], f32, tag="u")
        nc.vector.tensor_scalar_max(u[:], u1[:], 0.0)
        dap = stat.tile([128, RT], f32, tag="dap")
        nc.scalar.activation(dap[:], u[:], Act.Sqrt)
        # dist_an^2 = max(8*mn + 2, eps); >= 8 when row has no negatives.
        v1 = stat.tile([128, RT], f32, tag="v1")
        nc.vector.tensor_scalar(v1[:], mn[:], 8.0, 2.0, op0=Alu.mult, op1=Alu.add)
        v = stat.tile([128, RT], f32, tag="v")
        nc.vector.tensor_scalar_max(v[:], v1[:], EPS)
        sv = stat.tile([128, RT], f32, tag="sv")
        nc.scalar.activation(sv[:], v[:], Act.Sqrt)
        e = stat.tile([128, RT], f32, tag="e")
        nc.vector.tensor_scalar(e[:], v[:], 6.0, None, op0=Alu.is_gt)
        ome = stat.tile([128, RT], f32, tag="ome")
        nc.vector.tensor_scalar(ome[:], e[:], -1.0, 1.0, op0=Alu.mult, op1=Alu.add)
        t1 = stat.tile([128, RT], f32, tag="t1")
        nc.vector.tensor_tensor(t1[:], sv[:], ome[:], op=Alu.mult)
        dan = stat.tile([128, RT], f32, tag="dan")
        nc.vector.tensor_tensor(dan[:], t1[:], e[:], op=Alu.add)
        df = stat.tile([128, RT], f32, tag="df")
        nc.vector.tensor_tensor(df[:], dap[:], dan[:], op=Alu.subtract)
        lrow = stat.tile([128, RT], f32, tag="lrow")
        nc.vector.tensor_scalar(
            lrow[:], df[:], MARGIN, 0.0, op0=Alu.add, op1=Alu.max
        )

        # Row-sum across partitions via ones-matmul, then across row tiles.
        ps2 = psum.tile([1, RT], f32, tag="ps")
        nc.tensor.matmul(ps2[:], lhsT=ones[:], rhs=lrow[:], start=True, stop=True)
        tot = stat.tile([1, 1], f32, tag="tot")
        nc.vector.tensor_reduce(tot[:], ps2[:], axis=AxX, op=Alu.add)
        nc.sync.dma_start(out[:], tot[:])

    nc.compile()
    return nc


def _get_nc():
    if "nc" not in _CACHE:
        _CACHE["nc"] = _build()
    return _CACHE["nc"]


def make_in_maps(inputs: np.ndarray, targets: np.ndarray):
    inputs = np.ascontiguousarray(inputs, dtype=np.float32)
    tf = targets.astype(np.float32)
    in_maps = []
    for r in range(NCORES):
        rows = slice(r * RPC, (r + 1) * RPC)
        opp = slice(HALF, N) if r * RPC < HALF else slice(0, HALF)
        in_maps.append(
            {
                # partition p holds rows 4p..4p+3 (contiguous 4KB DMA);
                # "tile" t within a group is row 4p+t.
                "xb": inputs[rows].reshape(128, RT * D),
                "xo": inputs[opp].reshape(128, OT * D),
                "tb": tf[rows].reshape(128, RT),
                # xo partition k holds rows 16k..16k+15; group n covers tile
                # slices 4n..4n+3, so distance column n*512 + i*128 + k is
                # xo-row 16k + 4n + i: permute targets to match.
                "to": tf[opp].reshape(128, 4, 4).transpose(1, 2, 0).reshape(1, HALF),
            }
        )
    return in_maps


def kernel(inputs: np.ndarray, targets: np.ndarray) -> np.ndarray:
    from concourse.bass_utils import run_bass_kernel_spmd

    nc = _get_nc()
    in_maps = make_in_maps(inputs, targets)
    res = run_bass_kernel_spmd(nc, in_maps, list(range(NCORES)))
    total = sum(float(res.results[i]["out"][0, 0]) for i in range(NCORES))
    return np.float32(total / N)



# revision 8
# speedup vs baseline: 1.2073x; 1.2073x over previous
"""Trainium2 Bass kernel for a hard-triplet margin-ranking loss.

Sharding: data-parallel over anchor rows. 8 cores x 512 rows each. Rows in
the first half of the batch mine over columns [2048:4096], rows in the second
half over [0:2048], so each core needs only its 512x2048 slice of the
distance matrix. Per core:

  1. Features arrive as fp16 (host-converted; 5e-4 relative quantization,
     far inside the 2e-2 tolerance) in five [128, 4x256] group tiles.
  2. Row norms via ACT Square+accum (fp32 accumulate); inv = 1/(sqrt(sq)+eps)
     (DVE reciprocal). Anchor rows are scaled by -0.25*inv, opposite rows by
     inv, so the PE matmul yields pm = -0.25*<xn_i, xn_j> in [-0.25, 0.25]
     and dist^2 = 2 + 8*pm.
  3. Normalize via DVE tensor_scalar in fp16 (4x DVE fast mode), PE-transpose
     per group (fp16 matmul: 1 cycle/row vs 4 for fp32), evacuate PSUM per
     group with one ACT copy into fp16 xT tiles.
  4. pm = xbT.T @ xoT on PE (fp16 inputs, fp32 PSUM accumulate, K=256).
  5. Same-class mask built once per row tile as mq = (t_o == t_b) in fp16
     (tensor_scalar is_equal, 4x fast mode; the opposite-half targets arrive
     host-pre-broadcast to [128, 2048] fp16 so no on-chip broadcast).
     One DVE tensor_tensor_reduce fuses PSUM evacuation + mask add + row max:
     w = pm + mq (fp16), accum = row-max -> hardest positive (matched columns
     land in [0.75, 1.25], unmatched in [-0.25, 0.25]).
  6. Row min -> hardest negative: two halving tensor_tensor(min) levels
     ([128,2048]->[128,512], 2x DVE fast mode on fp16 -- TensorReduce has no
     fast mode, so the tree is cheaper), then one small tensor_reduce.
  7. dist_ap = sqrt(relu(8*mx - 6)) (exact 0 when a row has no positives),
     dist_an = sqrt(max(8*mn + 2, eps)) or 1.0 when a row has no negatives
     (then 8*mn + 2 >= 8 > 6 >= any real dist^2),
     row loss = relu(dist_ap - dist_an + margin); ones-matmul row-sum.
  8. Host sums the 8 per-core partial sums / 4096.
"""

import numpy as np

N, D = 4096, 256
HALF = N // 2
NCORES = 8
RPC = N // NCORES  # 512 anchor rows per core
RT = RPC // 128    # 4 anchor row tiles
OT = HALF // 128   # 16 opposite-half tiles
NT = RT + OT       # 20 input tiles
NG = NT // 4       # 5 groups of 4 tiles
MARGIN = 0.3
EPS = 1e-6
S = 0.125          # anchor pre-scale: pm = -2*S*dot = -0.25*dot

_CACHE = {}


def _build():
    from contextlib import ExitStack

    import concourse.bacc as bacc
    import concourse.bass as bass
    import concourse.tile as tile
    from concourse import masks, mybir

    f32 = mybir.dt.float32
    f16 = mybir.dt.float16
    Alu = mybir.AluOpType
    Act = mybir.ActivationFunctionType
    AxX = mybir.AxisListType.X
    ts = bass.ts

    nc = bacc.Bacc(
        "TRN2",
        target_bir_lowering=False,
        debug=False,
        enable_asserts=True,
        num_devices=NCORES,
    )
    xb = nc.dram_tensor("xb", [128, RT * D], f16, kind="ExternalInput").ap()
    xo = nc.dram_tensor("xo", [128, OT * D], f16, kind="ExternalInput").ap()
    tb = nc.dram_tensor("tb", [128, RT], f32, kind="ExternalInput").ap()
    to = nc.dram_tensor("to", [128, HALF], f16, kind="ExternalInput").ap()
    out = nc.dram_tensor("out", [1, 1], f32, kind="ExternalOutput").ap()

    with tile.TileContext(nc) as tc, ExitStack() as ctx:
        const = ctx.enter_context(tc.tile_pool(name="const", bufs=1))
        xin = ctx.enter_context(tc.tile_pool(name="xin", bufs=1))
        xt = ctx.enter_context(tc.tile_pool(name="xt", bufs=1))
        stat = ctx.enter_context(tc.tile_pool(name="stat", bufs=1))
        scr = ctx.enter_context(tc.tile_pool(name="scr", bufs=3))
        mqp = ctx.enter_context(tc.tile_pool(name="mqp", bufs=4))
        wide = ctx.enter_context(tc.tile_pool(name="wide", bufs=2))
        tree = ctx.enter_context(tc.tile_pool(name="tree", bufs=2))
        psum = ctx.enter_context(tc.tile_pool(name="psum", bufs=2, space="PSUM"))

        ident = const.tile([128, 128], f16, tag="ident")
        masks.make_identity(nc, ident[:])
        ones = const.tile([128, 1], f32, tag="ones")
        nc.vector.memset(ones[:], 1.0)

        # Targets: opposite-half row pre-broadcast on host to [128, HALF];
        # per-row targets as one [128, RT] per-partition scalar bank.
        tob = const.tile([128, HALF], f16, tag="tob")
        nc.sync.dma_start(tob[:], to[:])
        tbt = const.tile([128, RT], f32, tag="tbt")
        nc.sync.dma_start(tbt[:], tb[:])

        # Feature tiles in 5 groups of 4: group 0 = anchors, 1..4 = opposite.
        xg = []
        for g in range(NG):
            gt = xin.tile([128, 4 * D], f16, tag=f"xg{g}")
            if g == 0:
                nc.sync.dma_start(gt[:], xb[:])
            else:
                nc.sync.dma_start(gt[:], xo[:, (g - 1) * 4 * D : g * 4 * D])
            xg.append(gt)

        # Same-class masks, one per anchor row tile (independent of features,
        # so DVE can build them while ACT computes norms).
        mq = []
        for r in range(RT):
            m = mqp.tile([128, HALF], f16, tag=f"mq{r}")
            nc.vector.tensor_scalar(
                m[:], tob[:], tbt[:, r : r + 1], None, op0=Alu.is_equal
            )
            mq.append(m)

        # Row norms: sq[p, t] = sum_d x[p+128t, d]^2, one ACT op per tile.
        sq = stat.tile([128, NT], f32, tag="sq")
        for t in range(NT):
            s = scr.tile([128, D], f16, tag="sq_scratch")
            nc.scalar.activation(
                s[:], xg[t // 4][:, ts(t % 4, D)], Act.Square,
                accum_out=sq[:, t : t + 1],
            )
        nrm = stat.tile([128, NT], f32, tag="nrm")
        nc.scalar.activation(nrm[:], sq[:], Act.Sqrt)
        nrme = stat.tile([128, NT], f32, tag="nrme")
        nc.vector.tensor_scalar_add(nrme[:], nrm[:], EPS)
        inv = stat.tile([128, NT], f32, tag="inv")
        nc.vector.reciprocal(inv[:], nrme[:])
        inv2 = stat.tile([128, RT], f32, tag="inv2")
        nc.vector.tensor_scalar_mul(inv2[:], inv[:, 0:RT], -2.0 * S)

        # Normalize (DVE fp16 4x fast mode) + PE-transpose per group;
        # evacuate with one ACT copy per group.  Group g tile layout:
        # [128 dims(c), 512 rows] at columns [c*512, (c+1)*512).
        xT = []
        for g in range(NG):
            gt = xt.tile([128, 1024], f16, tag=f"xT{g}")
            pt = psum.tile([128, 1024], f16, tag="ps")
            for i in range(4):
                t = g * 4 + i
                xn = scr.tile([128, D], f16, tag="xn")
                sc = inv2[:, t : t + 1] if t < RT else inv[:, t : t + 1]
                nc.vector.tensor_scalar_mul(xn[:], xg[g][:, ts(i, D)], sc)
                for c in range(2):
                    nc.tensor.transpose(
                        pt[:, ts(c * 4 + i, 128)], xn[:, ts(c, 128)], ident[:]
                    )
            nc.scalar.copy(gt[:], pt[:])
            xT.append(gt)

        # Main matmul; fused PSUM-evac + mask add + row max via one
        # tensor_tensor_reduce; row min via Pool halving tree + small DVE
        # reduce.  Per anchor row tile.
        mx = stat.tile([128, RT], f32, tag="mx")
        mn = stat.tile([128, RT], f32, tag="mn")
        for r in range(RT):
            pm = psum.tile([128, 2048], f32, tag="ps")
            for n in range(4):
                for c in range(2):
                    nc.tensor.matmul(
                        pm[:, ts(n, 512)],
                        lhsT=xT[0][:, c * RPC + r * 128 : c * RPC + (r + 1) * 128],
                        rhs=xT[1 + n][:, ts(c, 512)],
                        start=(c == 0),
                        stop=(c == 1),
                    )
            w = wide.tile([128, HALF], f16, tag="w")
            nc.vector.scalar_tensor_tensor(
                w[:], tob[:], tbt[:, r : r + 1], pm[:],
                op0=Alu.is_equal, op1=Alu.add,
            )
            nc.vector.tensor_reduce(mx[:, r : r + 1], w[:], axis=AxX, op=Alu.max)
            t1 = tree.tile([128, 1024], f16, tag="t1")
            nc.vector.tensor_tensor(t1[:], w[:, 0:1024], w[:, 1024:2048], op=Alu.min)
            t2 = tree.tile([128, 512], f16, tag="t2")
            nc.vector.tensor_tensor(t2[:], t1[:, 0:512], t1[:, 512:1024], op=Alu.min)
            nc.vector.tensor_reduce(mn[:, r : r + 1], t2[:], axis=AxX, op=Alu.min)

        # Epilogue on [128, RT] (relu/sqrt chains on ACT to relieve DVE).
        bm6 = const.tile([128, 1], f32, tag="bm6")
        nc.vector.memset(bm6[:], -6.0)
        bp2 = const.tile([128, 1], f32, tag="bp2")
        nc.vector.memset(bp2[:], 2.0)
        beps = const.tile([128, 1], f32, tag="beps")
        nc.vector.memset(beps[:], EPS)
        # dist_ap^2 = relu(8*mx - 6); exact 0 when row has no positives.
        u = stat.tile([128, RT], f32, tag="u")
        nc.scalar.activation(u[:], mx[:], Act.Relu, bias=bm6[:], scale=8.0)
        dap = stat.tile([128, RT], f32, tag="dap")
        nc.scalar.activation(dap[:], u[:], Act.Sqrt)
        # dist_an^2 = relu(8*mn + 2) + eps; >= 8 when row has no negatives.
        v = stat.tile([128, RT], f32, tag="v")
        nc.scalar.activation(v[:], mn[:], Act.Relu, bias=bp2[:], scale=8.0)
        sv = stat.tile([128, RT], f32, tag="sv")
        nc.scalar.activation(sv[:], v[:], Act.Sqrt, bias=beps[:])
        # no-negatives flag: 8*mn + 2 > 6  <=>  mn > 0.5
        e = stat.tile([128, RT], f32, tag="e")
        nc.vector.tensor_scalar(e[:], mn[:], 0.5, None, op0=Alu.is_gt)
        ome = stat.tile([128, RT], f32, tag="ome")
        nc.vector.tensor_scalar(ome[:], e[:], -1.0, 1.0, op0=Alu.mult, op1=Alu.add)
        t1e = stat.tile([128, RT], f32, tag="t1e")
        nc.vector.tensor_tensor(t1e[:], sv[:], ome[:], op=Alu.mult)
        dan = stat.tile([128, RT], f32, tag="dan")
        nc.vector.tensor_tensor(dan[:], t1e[:], e[:], op=Alu.add)
        df = stat.tile([128, RT], f32, tag="df")
        nc.vector.tensor_tensor(df[:], dap[:], dan[:], op=Alu.subtract)
        lrow = stat.tile([128, RT], f32, tag="lrow")
        nc.vector.tensor_scalar(
            lrow[:], df[:], MARGIN, 0.0, op0=Alu.add, op1=Alu.max
        )

        # Row-sum across partitions via ones-matmul, then across row tiles.
        ps2 = psum.tile([1, RT], f32, tag="ps")
        nc.tensor.matmul(ps2[:], lhsT=ones[:], rhs=lrow[:], start=True, stop=True)
        tot = stat.tile([1, 1], f32, tag="tot")
        nc.vector.tensor_reduce(tot[:], ps2[:], axis=AxX, op=Alu.add)
        nc.sync.dma_start(out[:], tot[:])

    nc.compile()
    return nc


def _get_nc():
    if "nc" not in _CACHE:
        _CACHE["nc"] = _build()
    return _CACHE["nc"]


def make_in_maps(inputs: np.ndarray, targets: np.ndarray):
    x16 = np.ascontiguousarray(inputs, dtype=np.float32).astype(np.float16)
    tf = targets.astype(np.float32)
    t16 = targets.astype(np.float16)
    in_maps = []
    for r in range(NCORES):
        rows = slice(r * RPC, (r + 1) * RPC)
        opp = slice(HALF, N) if r * RPC < HALF else slice(0, HALF)
        # xo partition k holds rows 16k..16k+15; group n covers tile slices
        # 4n..4n+3, so distance column n*512 + i*128 + k is xo-row 16k + 4n
        # + i: permute targets to match, then pre-broadcast to all
        # partitions (replaces the on-chip GPSIMD broadcast with DMA).
        to_row = (
            t16[opp].reshape(128, 4, 4).transpose(1, 2, 0).reshape(1, HALF)
        )
        in_maps.append(
            {
                # partition p holds rows 4p..4p+3 (contiguous 2KB DMA);
                # "tile" t within a group is row 4p+t.
                "xb": x16[rows].reshape(128, RT * D),
                "xo": x16[opp].reshape(128, OT * D),
                "tb": tf[rows].reshape(128, RT),
                "to": np.ascontiguousarray(np.broadcast_to(to_row, (128, HALF))),
            }
        )
    return in_maps


def kernel(inputs: np.ndarray, targets: np.ndarray) -> np.ndarray:
    from concourse.bass_utils import run_bass_kernel_spmd

    nc = _get_nc()
    in_maps = make_in_maps(inputs, targets)
    res = run_bass_kernel_spmd(nc, in_maps, list(range(NCORES)))
    total = sum(float(res.results[i]["out"][0, 0]) for i in range(NCORES))
    return np.float32(total / N)


# revision 10
# speedup vs baseline: 1.3167x; 1.0906x over previous
"""Trainium2 Bass kernel for a hard-triplet margin-ranking loss.

Sharding: data-parallel over anchor rows. 8 cores x 512 rows each. Rows in
the first half of the batch mine over columns [2048:4096], rows in the second
half over [0:2048], so each core needs only its 512x2048 slice of the
distance matrix. Per core:

  1. Features arrive as fp16 (host-converted; 5e-4 relative quantization,
     far inside the 2e-2 tolerance) in five [128, 4x256] group tiles.
  2. Row norms via ACT Square+accum (fp32 accumulate); per-GROUP norm chains
     (sqrt/eps/recip on [128,4]) so each group normalizes as soon as its own
     four squares land instead of waiting for all twenty.
  3. Normalize via DVE tensor_scalar in fp16 (4x DVE fast mode; anchors
     scaled by -0.25*inv so pm = -0.25*<xn_i,xn_j>, dist^2 = 2 + 8*pm),
     PE-transpose per group (fp16: 1 cycle/row vs 4 for fp32), evacuate PSUM
     per group with one ACT copy into fp16 xT tiles.
  4. Same-class mask built once per row tile as mq = (t_o == t_b) in fp16
     (tensor_scalar is_equal, 4x fast mode; opposite-half targets arrive
     host-pre-broadcast to [128, 2048] fp16).
  5. pm = xbT.T @ xoT on PE (fp16, fp32 PSUM, K=256) PLUS a third K-chunk
     (1000*I).T @ mq_r that adds 1000*mask inside PSUM on the otherwise
     idle PE -- no separate DVE mask pass. Matched columns land at
     ~1000+pm, unmatched stay in [-0.25, 0.25].
  6. Row max (hardest positive) and row min (hardest negative) are two
     plain tensor_reduce ops straight out of PSUM.
  7. dist_ap = sqrt(relu(8*mx - 7998)) (exact 0 when a row has no
     positives), dist_an = sqrt(relu(8*mn + 2) + eps) or 1.0 when a row has
     no negatives (mn > 0.5 only when every column is same-class),
     row loss = relu(dist_ap - dist_an + margin); ones-matmul row-sum.
  8. Host sums the 8 per-core partial sums / 4096.
"""

import numpy as np

N, D = 4096, 256
HALF = N // 2
NCORES = 8
RPC = N // NCORES  # 512 anchor rows per core
RT = RPC // 128    # 4 anchor row tiles
OT = HALF // 128   # 16 opposite-half tiles
NT = RT + OT       # 20 input tiles
NG = NT // 4       # 5 groups of 4 tiles
MARGIN = 0.3
EPS = 1e-6
S = 0.125          # anchor pre-scale: pm = -2*S*dot = -0.25*dot
MQ = 1000.0        # mask offset added via the PE (exact in fp16)

_CACHE = {}


def _build():
    from contextlib import ExitStack

    import concourse.bacc as bacc
    import concourse.bass as bass
    import concourse.tile as tile
    from concourse import masks, mybir

    f32 = mybir.dt.float32
    f16 = mybir.dt.float16
    Alu = mybir.AluOpType
    Act = mybir.ActivationFunctionType
    AxX = mybir.AxisListType.X
    ts = bass.ts

    nc = bacc.Bacc(
        "TRN2",
        target_bir_lowering=False,
        debug=False,
        enable_asserts=True,
        num_devices=NCORES,
    )
    xb = nc.dram_tensor("xb", [128, RT * D], f16, kind="ExternalInput").ap()
    xo = nc.dram_tensor("xo", [128, OT * D], f16, kind="ExternalInput").ap()
    tb = nc.dram_tensor("tb", [128, RT], f32, kind="ExternalInput").ap()
    to = nc.dram_tensor("to", [128, HALF], f16, kind="ExternalInput").ap()
    out = nc.dram_tensor("out", [1, 1], f32, kind="ExternalOutput").ap()

    with tile.TileContext(nc) as tc, ExitStack() as ctx:
        const = ctx.enter_context(tc.tile_pool(name="const", bufs=1))
        xin = ctx.enter_context(tc.tile_pool(name="xin", bufs=1))
        xt = ctx.enter_context(tc.tile_pool(name="xt", bufs=1))
        stat = ctx.enter_context(tc.tile_pool(name="stat", bufs=1))
        scr = ctx.enter_context(tc.tile_pool(name="scr", bufs=3))
        mqp = ctx.enter_context(tc.tile_pool(name="mqp", bufs=4))
        psum = ctx.enter_context(tc.tile_pool(name="psum", bufs=2, space="PSUM"))

        ident = const.tile([128, 128], f16, tag="ident")
        masks.make_identity(nc, ident[:])
        # J = MQ * I in fp16: the third matmul K-chunk adds MQ*mask in PSUM.
        jmask = const.tile([128, 128], f16, tag="jmask")
        nc.gpsimd.memset(jmask[:], 0.0)
        nc.gpsimd.affine_select(
            out=jmask[:], in_=jmask[:], compare_op=Alu.not_equal,
            fill=MQ, base=0, pattern=[[-1, 128]], channel_multiplier=1,
        )
        ones = const.tile([128, 1], f32, tag="ones")
        nc.vector.memset(ones[:], 1.0)

        # Targets: opposite-half row pre-broadcast on host to [128, HALF];
        # per-row targets as one [128, RT] per-partition scalar bank.
        tob = const.tile([128, HALF], f16, tag="tob")
        nc.sync.dma_start(tob[:], to[:])
        tbt = const.tile([128, RT], f32, tag="tbt")
        nc.sync.dma_start(tbt[:], tb[:])

        # Feature tiles in 5 groups of 4: group 0 = anchors, 1..4 = opposite.
        xg = []
        for g in range(NG):
            gt = xin.tile([128, 4 * D], f16, tag=f"xg{g}")
            if g == 0:
                nc.sync.dma_start(gt[:], xb[:])
            else:
                nc.sync.dma_start(gt[:], xo[:, (g - 1) * 4 * D : g * 4 * D])
            xg.append(gt)

        # Same-class masks, one per anchor row tile (only needs tob/tbt, so
        # DVE can build them while ACT computes norms).
        mq = []
        for r in range(RT):
            m = mqp.tile([128, HALF], f16, tag=f"mq{r}")
            nc.vector.tensor_scalar(
                m[:], tob[:], tbt[:, r : r + 1], None, op0=Alu.is_equal
            )
            mq.append(m)

        # Per group: squares -> norm chain -> normalize -> transpose -> evac.
        # Group g tile layout: [128 dims(c), 512 rows] at cols [c*512,...).
        xT = []
        for g in range(NG):
            sqg = stat.tile([128, 4], f32, tag=f"sq{g}")
            for i in range(4):
                s = scr.tile([128, D], f16, tag="sq_scratch")
                nc.scalar.activation(
                    s[:], xg[g][:, ts(i, D)], Act.Square,
                    accum_out=sqg[:, i : i + 1],
                )
            nrm = stat.tile([128, 4], f32, tag=f"nrm{g}")
            nc.scalar.activation(nrm[:], sqg[:], Act.Sqrt)
            nrme = stat.tile([128, 4], f32, tag=f"nrme{g}")
            nc.vector.tensor_scalar_add(nrme[:], nrm[:], EPS)
            inv = stat.tile([128, 4], f32, tag=f"inv{g}")
            nc.vector.reciprocal(inv[:], nrme[:])
            if g == 0:
                inv2 = stat.tile([128, 4], f32, tag="inv2")
                nc.vector.tensor_scalar_mul(inv2[:], inv[:], -2.0 * S)
                sc_tile = inv2
            else:
                sc_tile = inv

            gt = xt.tile([128, 1024], f16, tag=f"xT{g}")
            pt = psum.tile([128, 1024], f16, tag="ps")
            for i in range(4):
                xn = scr.tile([128, D], f16, tag="xn")
                nc.vector.tensor_scalar_mul(
                    xn[:], xg[g][:, ts(i, D)], sc_tile[:, i : i + 1]
                )
                for c in range(2):
                    nc.tensor.transpose(
                        pt[:, ts(c * 4 + i, 128)], xn[:, ts(c, 128)], ident[:]
                    )
            nc.scalar.copy(gt[:], pt[:])
            xT.append(gt)

        # Main matmul with the mask folded in as a third K-chunk; row
        # max/min as two tensor_reduce straight out of PSUM.
        mx = stat.tile([128, RT], f32, tag="mx")
        mn = stat.tile([128, RT], f32, tag="mn")
        for r in range(RT):
            pm = psum.tile([128, 2048], f32, tag="ps")
            for n in range(4):
                for c in range(2):
                    nc.tensor.matmul(
                        pm[:, ts(n, 512)],
                        lhsT=xT[0][:, c * RPC + r * 128 : c * RPC + (r + 1) * 128],
                        rhs=xT[1 + n][:, ts(c, 512)],
                        start=(c == 0),
                        stop=False,
                    )
                nc.tensor.matmul(
                    pm[:, ts(n, 512)],
                    lhsT=jmask[:],
                    rhs=mq[r][:, ts(n, 512)],
                    start=False,
                    stop=True,
                )
            nc.vector.tensor_reduce(mx[:, r : r + 1], pm[:], axis=AxX, op=Alu.max)
            nc.vector.tensor_reduce(mn[:, r : r + 1], pm[:], axis=AxX, op=Alu.min)

        # Epilogue on [128, RT] (relu/sqrt chains on ACT to relieve DVE).
        bap = const.tile([128, 1], f32, tag="bap")
        nc.vector.memset(bap[:], -(8.0 * MQ - 2.0))
        bp2 = const.tile([128, 1], f32, tag="bp2")
        nc.vector.memset(bp2[:], 2.0)
        beps = const.tile([128, 1], f32, tag="beps")
        nc.vector.memset(beps[:], EPS)
        # dist_ap^2 = 2 + 8*(mx - MQ) = 8*mx - (8*MQ - 2); relu -> exact 0
        # when the row has no positives (then mx <= 0.26).
        u = stat.tile([128, RT], f32, tag="u")
        nc.scalar.activation(u[:], mx[:], Act.Relu, bias=bap[:], scale=8.0)
        dap = stat.tile([128, RT], f32, tag="dap")
        nc.scalar.activation(dap[:], u[:], Act.Sqrt)
        # dist_an^2 = relu(8*mn + 2) + eps; mn > 0.5 only when the row has
        # no negatives (all columns pushed to ~MQ).
        v = stat.tile([128, RT], f32, tag="v")
        nc.scalar.activation(v[:], mn[:], Act.Relu, bias=bp2[:], scale=8.0)
        sv = stat.tile([128, RT], f32, tag="sv")
        nc.scalar.activation(sv[:], v[:], Act.Sqrt, bias=beps[:])
        e = stat.tile([128, RT], f32, tag="e")
        nc.vector.tensor_scalar(e[:], mn[:], 0.5, None, op0=Alu.is_gt)
        ome = stat.tile([128, RT], f32, tag="ome")
        nc.vector.tensor_scalar(ome[:], e[:], -1.0, 1.0, op0=Alu.mult, op1=Alu.add)
        t1e = stat.tile([128, RT], f32, tag="t1e")
        nc.vector.tensor_tensor(t1e[:], sv[:], ome[:], op=Alu.mult)
        dan = stat.tile([128, RT], f32, tag="dan")
        nc.vector.tensor_tensor(dan[:], t1e[:], e[:], op=Alu.add)
        df = stat.tile([128, RT], f32, tag="df")
        nc.vector.tensor_tensor(df[:], dap[:], dan[:], op=Alu.subtract)
        lrow = stat.tile([128, RT], f32, tag="lrow")
        nc.vector.tensor_scalar(
            lrow[:], df[:], MARGIN, 0.0, op0=Alu.add, op1=Alu.max
        )

        # Row-sum across partitions via ones-matmul, then across row tiles.
        ps2 = psum.tile([1, RT], f32, tag="ps")
        nc.tensor.matmul(ps2[:], lhsT=ones[:], rhs=lrow[:], start=True, stop=True)
        tot = stat.tile([1, 1], f32, tag="tot")
        nc.vector.tensor_reduce(tot[:], ps2[:], axis=AxX, op=Alu.add)
        nc.sync.dma_start(out[:], tot[:])

    nc.compile()
    return nc


def _get_nc():
    if "nc" not in _CACHE:
        _CACHE["nc"] = _build()
    return _CACHE["nc"]


def make_in_maps(inputs: np.ndarray, targets: np.ndarray):
    x16 = np.ascontiguousarray(inputs, dtype=np.float32).astype(np.float16)
    tf = targets.astype(np.float32)
    t16 = targets.astype(np.float16)
    in_maps = []
    for r in range(NCORES):
        rows = slice(r * RPC, (r + 1) * RPC)
        opp = slice(HALF, N) if r * RPC < HALF else slice(0, HALF)
        # xo partition k holds rows 16k..16k+15; group n covers tile slices
        # 4n..4n+3, so distance column n*512 + i*128 + k is xo-row 16k + 4n
        # + i: permute targets to match, then pre-broadcast to all
        # partitions (replaces the on-chip GPSIMD broadcast with DMA).
        to_row = (
            t16[opp].reshape(128, 4, 4).transpose(1, 2, 0).reshape(1, HALF)
        )
        in_maps.append(
            {
                # partition p holds rows 4p..4p+3 (contiguous 2KB DMA);
                # "tile" t within a group is row 4p+t.
                "xb": x16[rows].reshape(128, RT * D),
                "xo": x16[opp].reshape(128, OT * D),
                "tb": tf[rows].reshape(128, RT),
                "to": np.ascontiguousarray(np.broadcast_to(to_row, (128, HALF))),
            }
        )
    return in_maps


def kernel(inputs: np.ndarray, targets: np.ndarray) -> np.ndarray:
    from concourse.bass_utils import run_bass_kernel_spmd

    nc = _get_nc()
    in_maps = make_in_maps(inputs, targets)
    res = run_bass_kernel_spmd(nc, in_maps, list(range(NCORES)))
    total = sum(float(res.results[i]["out"][0, 0]) for i in range(NCORES))
    return np.float32(total / N)


# revision 12
# speedup vs baseline: 1.7848x; 1.3555x over previous
"""Trainium2 Bass kernel for a hard-triplet margin-ranking loss.

Sharding: data-parallel over anchor rows. 8 cores x 512 rows each. Rows in
the first half of the batch mine over columns [2048:4096], rows in the second
half over [0:2048], so each core needs only its 512x2048 slice of the
distance matrix. Per core:

  1. Features arrive as fp16 (host-converted; 5e-4 relative quantization,
     far inside the 2e-2 tolerance) in five [128, 4x256] group tiles.
  2. Row norms per group: ONE wide ACT Square into fp16 scratch, then four
     4x-fast-mode DVE tensor_scalar add-accumulates (one per 256-wide tile),
     ACT sqrt on [128,4], DVE eps+reciprocal. Per-group chains let each
     group normalize as soon as its own data lands.
  3. Normalize via DVE tensor_scalar in fp16 (4x fast mode; anchors scaled
     by -0.25*inv so pm = -0.25*<xn_i,xn_j>, dist^2 = 2 + 8*pm),
     PE-transpose per group (fp16: 1 cycle/row vs 4 for fp32), evacuate
     PSUM per group with one ACT copy into fp16 xT tiles.
  4. Same-class mask built once per row tile as mq = (t_o == t_b) in fp16
     (tensor_scalar is_equal, 4x fast mode; opposite-half targets arrive
     host-pre-broadcast to [128, 2048] fp16).
  5. pm = xbT.T @ xoT on PE (fp16, fp32 PSUM, K=256) PLUS a third K-chunk
     I.T @ mq_r that adds the +1 same-class mask inside PSUM on the
     otherwise idle PE -- no separate DVE mask pass. Matched columns land
     in [0.75, 1.25], unmatched in [-0.25, 0.25].
  6. Row max (hardest positive) and row min (hardest negative), each as a
     tensor_tensor halving level straight out of PSUM (cost = output width,
     so the 2048 inputs cost 1024 cycles) followed by a 4x-fast-mode
     tensor_scalar accumulate-reduce on the fp16 [128,1024] intermediate.
  7. dist_ap = sqrt(relu(8*mx - 6)) (exact 0 when a row has no positives),
     dist_an = sqrt(relu(8*mn + 2) + eps) or 1.0 when a row has no
     negatives (mn > 0.5 only when every column is same-class),
     row loss = relu(dist_ap - dist_an + margin); ones-matmul row-sum.
  8. Host sums the 8 per-core partial sums / 4096.
"""

import numpy as np

N, D = 4096, 256
HALF = N // 2
NCORES = 8
RPC = N // NCORES  # 512 anchor rows per core
RT = RPC // 128    # 4 anchor row tiles
OT = HALF // 128   # 16 opposite-half tiles
NT = RT + OT       # 20 input tiles
NG = NT // 4       # 5 groups of 4 tiles
MARGIN = 0.3
EPS = 1e-6
S = 0.125          # anchor pre-scale: pm = -2*S*dot = -0.25*dot

_CACHE = {}


def _build():
    from contextlib import ExitStack

    import concourse.bacc as bacc
    import concourse.bass as bass
    import concourse.tile as tile
    from concourse import masks, mybir

    f32 = mybir.dt.float32
    f16 = mybir.dt.float16
    Alu = mybir.AluOpType
    Act = mybir.ActivationFunctionType
    AxX = mybir.AxisListType.X
    ts = bass.ts

    nc = bacc.Bacc(
        "TRN2",
        target_bir_lowering=False,
        debug=False,
        enable_asserts=True,
        num_devices=NCORES,
    )
    xb = nc.dram_tensor("xb", [128, RT * D], f16, kind="ExternalInput").ap()
    xo = nc.dram_tensor("xo", [128, OT * D], f16, kind="ExternalInput").ap()
    tb = nc.dram_tensor("tb", [128, RT], f32, kind="ExternalInput").ap()
    to = nc.dram_tensor("to", [128, HALF], f16, kind="ExternalInput").ap()
    out = nc.dram_tensor("out", [1, 1], f32, kind="ExternalOutput").ap()

    with tile.TileContext(nc) as tc, ExitStack() as ctx:
        const = ctx.enter_context(tc.tile_pool(name="const", bufs=1))
        xin = ctx.enter_context(tc.tile_pool(name="xin", bufs=1))
        xt = ctx.enter_context(tc.tile_pool(name="xt", bufs=1))
        stat = ctx.enter_context(tc.tile_pool(name="stat", bufs=1))
        scr = ctx.enter_context(tc.tile_pool(name="scr", bufs=3))
        wscr = ctx.enter_context(tc.tile_pool(name="wscr", bufs=2))
        mqp = ctx.enter_context(tc.tile_pool(name="mqp", bufs=4))
        tree = ctx.enter_context(tc.tile_pool(name="tree", bufs=2))
        psum = ctx.enter_context(tc.tile_pool(name="psum", bufs=2, space="PSUM"))

        ident = const.tile([128, 128], f16, tag="ident")
        masks.make_identity(nc, ident[:])
        ones = const.tile([128, 1], f32, tag="ones")
        nc.vector.memset(ones[:], 1.0)

        # Targets: opposite-half row pre-broadcast on host to [128, HALF];
        # per-row targets as one [128, RT] per-partition scalar bank.
        tob = const.tile([128, HALF], f16, tag="tob")
        nc.sync.dma_start(tob[:], to[:])
        tbt = const.tile([128, RT], f32, tag="tbt")
        nc.sync.dma_start(tbt[:], tb[:])

        # Feature tiles in 5 groups of 4: group 0 = anchors, 1..4 = opposite.
        xg = []
        for g in range(NG):
            gt = xin.tile([128, 4 * D], f16, tag=f"xg{g}")
            if g == 0:
                nc.sync.dma_start(gt[:], xb[:])
            else:
                nc.sync.dma_start(gt[:], xo[:, (g - 1) * 4 * D : g * 4 * D])
            xg.append(gt)

        # Per group: wide square -> per-tile sum -> norm chain -> normalize
        # -> transpose -> evac.  Group g tile layout after transpose:
        # [128 dims(c), 512 rows] at columns [c*512, (c+1)*512).
        xT = []
        for g in range(NG):
            s2 = wscr.tile([128, 1024], f16, tag="s2")
            nc.scalar.activation(s2[:], xg[g][:], Act.Square)
            sqg = stat.tile([128, 4], f32, tag=f"sq{g}")
            for i in range(4):
                sd = scr.tile([128, D], f16, tag="sq_dummy")
                nc.vector.tensor_scalar(
                    sd[:], s2[:, ts(i, D)], 1.0, None,
                    op0=Alu.mult, op1=Alu.add, accum_out=sqg[:, i : i + 1],
                )
            nrm = stat.tile([128, 4], f32, tag=f"nrm{g}")
            nc.scalar.activation(nrm[:], sqg[:], Act.Sqrt)
            nrme = stat.tile([128, 4], f32, tag=f"nrme{g}")
            nc.vector.tensor_scalar_add(nrme[:], nrm[:], EPS)
            inv = stat.tile([128, 4], f32, tag=f"inv{g}")
            nc.vector.reciprocal(inv[:], nrme[:])
            if g == 0:
                inv2 = stat.tile([128, 4], f32, tag="inv2")
                nc.vector.tensor_scalar_mul(inv2[:], inv[:], -2.0 * S)
                sc_tile = inv2
            else:
                sc_tile = inv

            gt = xt.tile([128, 1024], f16, tag=f"xT{g}")
            pt = psum.tile([128, 1024], f16, tag="ps")
            for i in range(4):
                xn = scr.tile([128, D], f16, tag="xn")
                nc.vector.tensor_scalar_mul(
                    xn[:], xg[g][:, ts(i, D)], sc_tile[:, i : i + 1]
                )
                for c in range(2):
                    nc.tensor.transpose(
                        pt[:, ts(c * 4 + i, 128)], xn[:, ts(c, 128)], ident[:]
                    )
            nc.scalar.copy(gt[:], pt[:])
            xT.append(gt)

        # Same-class masks, one per anchor row tile (4x fast mode).
        mq = []
        for r in range(RT):
            m = mqp.tile([128, HALF], f16, tag=f"mq{r}")
            nc.vector.tensor_scalar(
                m[:], tob[:], tbt[:, r : r + 1], None, op0=Alu.is_equal
            )
            mq.append(m)

        # Main matmul with the +1 mask folded in as a third K-chunk (lhsT =
        # identity); row max/min via TT halving from PSUM + 4x TS-accum.
        mx = stat.tile([128, RT], f32, tag="mx")
        mn = stat.tile([128, RT], f32, tag="mn")
        for r in range(RT):
            pm = psum.tile([128, 2048], f32, tag="ps")
            for n in range(4):
                for c in range(2):
                    nc.tensor.matmul(
                        pm[:, ts(n, 512)],
                        lhsT=xT[0][:, c * RPC + r * 128 : c * RPC + (r + 1) * 128],
                        rhs=xT[1 + n][:, ts(c, 512)],
                        start=(c == 0),
                        stop=False,
                    )
                nc.tensor.matmul(
                    pm[:, ts(n, 512)],
                    lhsT=ident[:],
                    rhs=mq[r][:, ts(n, 512)],
                    start=False,
                    stop=True,
                )
            # One PSUM-read pass: evac to fp16 AND row-max accumulate; then
            # the min re-reads the fp16 SBUF copy at the 4x fast mode.
            w16 = tree.tile([128, HALF], f16, tag="w16")
            nc.vector.tensor_scalar(
                w16[:], pm[:], 1.0, None,
                op0=Alu.mult, op1=Alu.max, accum_out=mx[:, r : r + 1],
            )
            dn = tree.tile([128, HALF], f16, tag="dn")
            nc.vector.tensor_scalar(
                dn[:], w16[:], 1.0, None,
                op0=Alu.mult, op1=Alu.min, accum_out=mn[:, r : r + 1],
            )

        # Epilogue on [128, RT] (relu/sqrt chains on ACT to relieve DVE).
        bm6 = const.tile([128, 1], f32, tag="bm6")
        nc.vector.memset(bm6[:], -6.0)
        bp2 = const.tile([128, 1], f32, tag="bp2")
        nc.vector.memset(bp2[:], 2.0)
        beps = const.tile([128, 1], f32, tag="beps")
        nc.vector.memset(beps[:], EPS)
        # dist_ap^2 = relu(8*mx - 6); exact 0 when row has no positives.
        u = stat.tile([128, RT], f32, tag="u")
        nc.scalar.activation(u[:], mx[:], Act.Relu, bias=bm6[:], scale=8.0)
        dap = stat.tile([128, RT], f32, tag="dap")
        nc.scalar.activation(dap[:], u[:], Act.Sqrt)
        # dist_an^2 = relu(8*mn + 2) + eps; mn > 0.5 only when the row has
        # no negatives (all columns masked up by +1).
        v = stat.tile([128, RT], f32, tag="v")
        nc.scalar.activation(v[:], mn[:], Act.Relu, bias=bp2[:], scale=8.0)
        sv = stat.tile([128, RT], f32, tag="sv")
        nc.scalar.activation(sv[:], v[:], Act.Sqrt, bias=beps[:])
        e = stat.tile([128, RT], f32, tag="e")
        nc.vector.tensor_scalar(e[:], mn[:], 0.5, None, op0=Alu.is_gt)
        ome = stat.tile([128, RT], f32, tag="ome")
        nc.vector.tensor_scalar(ome[:], e[:], -1.0, 1.0, op0=Alu.mult, op1=Alu.add)
        t1e = stat.tile([128, RT], f32, tag="t1e")
        nc.vector.tensor_tensor(t1e[:], sv[:], ome[:], op=Alu.mult)
        dan = stat.tile([128, RT], f32, tag="dan")
        nc.vector.tensor_tensor(dan[:], t1e[:], e[:], op=Alu.add)
        df = stat.tile([128, RT], f32, tag="df")
        nc.vector.tensor_tensor(df[:], dap[:], dan[:], op=Alu.subtract)
        lrow = stat.tile([128, RT], f32, tag="lrow")
        nc.vector.tensor_scalar(
            lrow[:], df[:], MARGIN, 0.0, op0=Alu.add, op1=Alu.max
        )

        # Row-sum across partitions via ones-matmul, then across row tiles.
        ps2 = psum.tile([1, RT], f32, tag="ps")
        nc.tensor.matmul(ps2[:], lhsT=ones[:], rhs=lrow[:], start=True, stop=True)
        tot = stat.tile([1, 1], f32, tag="tot")
        nc.vector.tensor_reduce(tot[:], ps2[:], axis=AxX, op=Alu.add)
        nc.sync.dma_start(out[:], tot[:])

    nc.compile()
    return nc


def _get_nc():
    if "nc" not in _CACHE:
        _CACHE["nc"] = _build()
    return _CACHE["nc"]


def make_in_maps(inputs: np.ndarray, targets: np.ndarray):
    x16 = np.ascontiguousarray(inputs, dtype=np.float32).astype(np.float16)
    tf = targets.astype(np.float32)
    t16 = targets.astype(np.float16)
    in_maps = []
    for r in range(NCORES):
        rows = slice(r * RPC, (r + 1) * RPC)
        opp = slice(HALF, N) if r * RPC < HALF else slice(0, HALF)
        # xo partition k holds rows 16k..16k+15; group n covers tile slices
        # 4n..4n+3, so distance column n*512 + i*128 + k is xo-row 16k + 4n
        # + i: permute targets to match, then pre-broadcast to all
        # partitions (replaces the on-chip GPSIMD broadcast with DMA).
        to_row = (
            t16[opp].reshape(128, 4, 4).transpose(1, 2, 0).reshape(1, HALF)
        )
        in_maps.append(
            {
                # partition p holds rows 4p..4p+3 (contiguous 2KB DMA);
                # "tile" t within a group is row 4p+t.
                "xb": x16[rows].reshape(128, RT * D),
                "xo": x16[opp].reshape(128, OT * D),
                "tb": tf[rows].reshape(128, RT),
                "to": np.ascontiguousarray(np.broadcast_to(to_row, (128, HALF))),
            }
        )
    return in_maps


def kernel(inputs: np.ndarray, targets: np.ndarray) -> np.ndarray:
    from concourse.bass_utils import run_bass_kernel_spmd

    nc = _get_nc()
    in_maps = make_in_maps(inputs, targets)
    res = run_bass_kernel_spmd(nc, in_maps, list(range(NCORES)))
    total = sum(float(res.results[i]["out"][0, 0]) for i in range(NCORES))
    return np.float32(total / N)


# revision 15
# speedup vs baseline: 1.8846x; 1.0559x over previous
"""Trainium2 Bass kernel for a hard-triplet margin-ranking loss.

Sharding: data-parallel over anchor rows. 8 cores x 512 rows each. Rows in
the first half of the batch mine over columns [2048:4096], rows in the second
half over [0:2048], so each core needs only its 512x2048 slice of the
distance matrix. Per core:

  1. Features arrive as fp16 (host-converted; 5e-4 relative quantization,
     far inside the 2e-2 tolerance) in five [128, 4x256] group tiles.
  2. Row norms per group: ONE wide ACT Square into fp16 scratch, then four
     4x-fast-mode DVE tensor_scalar add-accumulates (one per 256-wide tile),
     ACT sqrt on [128,4], DVE eps+reciprocal. Per-group chains let each
     group normalize as soon as its own data lands.
  3. Normalize via DVE tensor_scalar in fp16 (4x fast mode; anchors scaled
     by -0.25*inv so pm = -0.25*<xn_i,xn_j>, dist^2 = 2 + 8*pm),
     PE-transpose per group (fp16: 1 cycle/row vs 4 for fp32), evacuate
     PSUM per group with one ACT copy into fp16 xT tiles.
  4. Same-class mask built once per row tile as mq = (t_o == t_b) in fp16
     (tensor_scalar is_equal, 4x fast mode; opposite-half targets arrive
     host-pre-broadcast to [128, 2048] fp16).
  5. pm = xbT.T @ xoT on PE (fp16, fp32 PSUM, K=256) PLUS a third K-chunk
     I.T @ mq_r that adds the +1 same-class mask inside PSUM on the
     otherwise idle PE -- no separate DVE mask pass. Matched columns land
     in [0.75, 1.25], unmatched in [-0.25, 0.25].
  6. Row max (hardest positive) and row min (hardest negative), each as a
     tensor_tensor halving level straight out of PSUM (cost = output width,
     so the 2048 inputs cost 1024 cycles) followed by a 4x-fast-mode
     tensor_scalar accumulate-reduce on the fp16 [128,1024] intermediate.
  7. dist_ap = sqrt(relu(8*mx - 6)) (exact 0 when a row has no positives),
     dist_an = sqrt(relu(8*mn + 2) + eps) or 1.0 when a row has no
     negatives (mn > 0.5 only when every column is same-class),
     row loss = relu(dist_ap - dist_an + margin); ones-matmul row-sum.
  8. Host sums the 8 per-core partial sums / 4096.
"""

import numpy as np

N, D = 4096, 256
HALF = N // 2
NCORES = 8
RPC = N // NCORES  # 512 anchor rows per core
RT = RPC // 128    # 4 anchor row tiles
OT = HALF // 128   # 16 opposite-half tiles
NT = RT + OT       # 20 input tiles
NG = NT // 4       # 5 groups of 4 tiles
MARGIN = 0.3
EPS = 1e-6
S = 0.125          # anchor pre-scale: pm = -2*S*dot = -0.25*dot

_CACHE = {}


def _build():
    from contextlib import ExitStack

    import concourse.bacc as bacc
    import concourse.bass as bass
    import concourse.tile as tile
    from concourse import masks, mybir

    f32 = mybir.dt.float32
    f16 = mybir.dt.float16
    Alu = mybir.AluOpType
    Act = mybir.ActivationFunctionType
    AxX = mybir.AxisListType.X
    ts = bass.ts

    nc = bacc.Bacc(
        "TRN2",
        target_bir_lowering=False,
        debug=False,
        enable_asserts=True,
        num_devices=NCORES,
    )
    xb = nc.dram_tensor("xb", [128, RT * D], f16, kind="ExternalInput").ap()
    xo = nc.dram_tensor("xo", [128, OT * D], f16, kind="ExternalInput").ap()
    tb = nc.dram_tensor("tb", [128, RT], f32, kind="ExternalInput").ap()
    to = nc.dram_tensor("to", [128, HALF], f16, kind="ExternalInput").ap()
    out = nc.dram_tensor("out", [1, 1], f32, kind="ExternalOutput").ap()

    with tile.TileContext(nc) as tc, ExitStack() as ctx:
        const = ctx.enter_context(tc.tile_pool(name="const", bufs=1))
        xin = ctx.enter_context(tc.tile_pool(name="xin", bufs=1))
        xt = ctx.enter_context(tc.tile_pool(name="xt", bufs=1))
        stat = ctx.enter_context(tc.tile_pool(name="stat", bufs=1))
        scr = ctx.enter_context(tc.tile_pool(name="scr", bufs=3))
        wscr = ctx.enter_context(tc.tile_pool(name="wscr", bufs=2))
        mqp = ctx.enter_context(tc.tile_pool(name="mqp", bufs=4))
        tree = ctx.enter_context(tc.tile_pool(name="tree", bufs=2))
        psum = ctx.enter_context(tc.tile_pool(name="psum", bufs=2, space="PSUM"))

        ident = const.tile([128, 128], f16, tag="ident")
        masks.make_identity(nc, ident[:])
        ones = const.tile([128, 1], f32, tag="ones")
        nc.vector.memset(ones[:], 1.0)

        # Warm both ACT function-table sets (Square/Copy and Sqrt/Relu)
        # during the DMA shadow so no LoadActFuncSet lands mid-pipeline.
        warm = const.tile([128, 1], f32, tag="warm")
        nc.scalar.activation(warm[:], ones[:], Act.Square)
        nc.scalar.activation(warm[:], ones[:], Act.Sqrt)
        nc.scalar.activation(warm[:], ones[:], Act.Relu)

        # Feature tiles in 5 groups of 4: group 0 = anchors, 1..4 =
        # opposite.  Anchors + targets on the SP DGE queue, opposite-half
        # groups on the Pool DGE queue so the first group isn't stuck
        # behind 1.5MB of queue-serial transfers.
        xg = []
        for g in range(NG):
            gt = xin.tile([128, 4 * D], f16, tag=f"xg{g}")
            if g == 0:
                nc.sync.dma_start(gt[:], xb[:])
            else:
                nc.gpsimd.dma_start(gt[:], xo[:, (g - 1) * 4 * D : g * 4 * D])
            xg.append(gt)

        # Targets: opposite-half row pre-broadcast on host to [128, HALF];
        # per-row targets as one [128, RT] per-partition scalar bank.
        tbt = const.tile([128, RT], f32, tag="tbt")
        nc.sync.dma_start(tbt[:], tb[:])
        tob = const.tile([128, HALF], f16, tag="tob")
        nc.sync.dma_start(tob[:], to[:])

        # Per group: wide square -> per-tile sum -> norm chain -> normalize
        # -> transpose -> evac.  Group g tile layout after transpose:
        # [128 dims(c), 512 rows] at columns [c*512, (c+1)*512).
        xT = []
        for g in range(NG):
            s2 = wscr.tile([128, 1024], f16, tag="s2")
            nc.scalar.activation(s2[:], xg[g][:], Act.Square)
            sqg = stat.tile([128, 4], f32, tag=f"sq{g}")
            for i in range(4):
                sd = scr.tile([128, D], f16, tag="sq_dummy")
                nc.vector.tensor_scalar(
                    sd[:], s2[:, ts(i, D)], 1.0, None,
                    op0=Alu.mult, op1=Alu.add, accum_out=sqg[:, i : i + 1],
                )
            nrm = stat.tile([128, 4], f32, tag=f"nrm{g}")
            nc.scalar.activation(nrm[:], sqg[:], Act.Sqrt)
            nrme = stat.tile([128, 4], f32, tag=f"nrme{g}")
            nc.vector.tensor_scalar_add(nrme[:], nrm[:], EPS)
            inv = stat.tile([128, 4], f32, tag=f"inv{g}")
            nc.vector.reciprocal(inv[:], nrme[:])
            if g == 0:
                inv2 = stat.tile([128, 4], f32, tag="inv2")
                nc.vector.tensor_scalar_mul(inv2[:], inv[:], -2.0 * S)
                sc_tile = inv2
            else:
                sc_tile = inv

            gt = xt.tile([128, 1024], f16, tag=f"xT{g}")
            pt = psum.tile([128, 1024], f16, tag="ps")
            for i in range(4):
                xn = scr.tile([128, D], f16, tag="xn")
                nc.vector.tensor_scalar_mul(
                    xn[:], xg[g][:, ts(i, D)], sc_tile[:, i : i + 1]
                )
                for c in range(2):
                    nc.tensor.transpose(
                        pt[:, ts(c * 4 + i, 128)], xn[:, ts(c, 128)], ident[:]
                    )
            nc.scalar.copy(gt[:], pt[:])
            xT.append(gt)

        # Same-class masks, one per anchor row tile (4x fast mode).
        mq = []
        for r in range(RT):
            m = mqp.tile([128, HALF], f16, tag=f"mq{r}")
            nc.vector.tensor_scalar(
                m[:], tob[:], tbt[:, r : r + 1], None, op0=Alu.is_equal
            )
            mq.append(m)

        # Epilogue tiles + bias constants (epilogue runs in two halves so
        # tiles 0-1 decode while tiles 2-3 are still reducing).
        bm6 = const.tile([128, 1], f32, tag="bm6")
        nc.vector.memset(bm6[:], -6.0)
        bp2 = const.tile([128, 1], f32, tag="bp2")
        nc.vector.memset(bp2[:], 2.0)
        beps = const.tile([128, 1], f32, tag="beps")
        nc.vector.memset(beps[:], EPS)
        mx = stat.tile([128, RT], f32, tag="mx")
        mn = stat.tile([128, RT], f32, tag="mn")
        u = stat.tile([128, RT], f32, tag="u")
        dap = stat.tile([128, RT], f32, tag="dap")
        v = stat.tile([128, RT], f32, tag="v")
        sv = stat.tile([128, RT], f32, tag="sv")
        e = stat.tile([128, RT], f32, tag="e")
        ome = stat.tile([128, RT], f32, tag="ome")
        t1e = stat.tile([128, RT], f32, tag="t1e")
        dan = stat.tile([128, RT], f32, tag="dan")
        df = stat.tile([128, RT], f32, tag="df")
        lrow = stat.tile([128, RT], f32, tag="lrow")

        def epilogue(sl):
            # dist_ap^2 = relu(8*mx - 6); exact 0 when row has no
            # positives.  dist_an^2 = relu(8*mn + 2) + eps; mn > 0.5 only
            # when the row has no negatives (all columns masked up by +1).
            nc.scalar.activation(u[:, sl], mx[:, sl], Act.Relu, bias=bm6[:], scale=8.0)
            nc.scalar.activation(dap[:, sl], u[:, sl], Act.Sqrt)
            nc.scalar.activation(v[:, sl], mn[:, sl], Act.Relu, bias=bp2[:], scale=8.0)
            nc.scalar.activation(sv[:, sl], v[:, sl], Act.Sqrt, bias=beps[:])
            nc.vector.tensor_scalar(e[:, sl], mn[:, sl], 0.5, None, op0=Alu.is_gt)
            nc.vector.tensor_scalar(
                ome[:, sl], e[:, sl], -1.0, 1.0, op0=Alu.mult, op1=Alu.add
            )
            nc.vector.tensor_tensor(t1e[:, sl], sv[:, sl], ome[:, sl], op=Alu.mult)
            nc.vector.tensor_tensor(dan[:, sl], t1e[:, sl], e[:, sl], op=Alu.add)
            nc.vector.tensor_tensor(df[:, sl], dap[:, sl], dan[:, sl], op=Alu.subtract)
            nc.vector.tensor_scalar(
                lrow[:, sl], df[:, sl], MARGIN, 0.0, op0=Alu.add, op1=Alu.max
            )

        # Main matmul with the +1 mask folded in as a third K-chunk (lhsT =
        # identity); row max via the PSUM-evac pass, min via 4x TS-accum.
        for r in range(RT):
            pm = psum.tile([128, 2048], f32, tag="ps")
            for n in range(4):
                for c in range(2):
                    nc.tensor.matmul(
                        pm[:, ts(n, 512)],
                        lhsT=xT[0][:, c * RPC + r * 128 : c * RPC + (r + 1) * 128],
                        rhs=xT[1 + n][:, ts(c, 512)],
                        start=(c == 0),
                        stop=False,
                    )
                nc.tensor.matmul(
                    pm[:, ts(n, 512)],
                    lhsT=ident[:],
                    rhs=mq[r][:, ts(n, 512)],
                    start=False,
                    stop=True,
                )
            # One PSUM-read pass: evac to fp16 AND row-max accumulate; then
            # the min re-reads the fp16 SBUF copy at the 4x fast mode.
            w16 = tree.tile([128, HALF], f16, tag="w16")
            nc.vector.tensor_scalar(
                w16[:], pm[:], 1.0, None,
                op0=Alu.mult, op1=Alu.max, accum_out=mx[:, r : r + 1],
            )
            dn = tree.tile([128, HALF], f16, tag="dn")
            nc.vector.tensor_scalar(
                dn[:], w16[:], 1.0, None,
                op0=Alu.mult, op1=Alu.min, accum_out=mn[:, r : r + 1],
            )
            if r == 1:
                epilogue(slice(0, 2))
        epilogue(slice(2, RT))

        # Row-sum across partitions via ones-matmul, then across row tiles.
        ps2 = psum.tile([1, RT], f32, tag="ps")
        nc.tensor.matmul(ps2[:], lhsT=ones[:], rhs=lrow[:], start=True, stop=True)
        tot = stat.tile([1, 1], f32, tag="tot")
        nc.vector.tensor_reduce(tot[:], ps2[:], axis=AxX, op=Alu.add)
        nc.sync.dma_start(out[:], tot[:])

    nc.compile()
    return nc


def _get_nc():
    if "nc" not in _CACHE:
        _CACHE["nc"] = _build()
    return _CACHE["nc"]


def make_in_maps(inputs: np.ndarray, targets: np.ndarray):
    x16 = np.ascontiguousarray(inputs, dtype=np.float32).astype(np.float16)
    tf = targets.astype(np.float32)
    t16 = targets.astype(np.float16)
    in_maps = []
    for r in range(NCORES):
        rows = slice(r * RPC, (r + 1) * RPC)
        opp = slice(HALF, N) if r * RPC < HALF else slice(0, HALF)
        # xo partition k holds rows 16k..16k+15; group n covers tile slices
        # 4n..4n+3, so distance column n*512 + i*128 + k is xo-row 16k + 4n
        # + i: permute targets to match, then pre-broadcast to all
        # partitions (replaces the on-chip GPSIMD broadcast with DMA).
        to_row = (
            t16[opp].reshape(128, 4, 4).transpose(1, 2, 0).reshape(1, HALF)
        )
        in_maps.append(
            {
                # partition p holds rows 4p..4p+3 (contiguous 2KB DMA);
                # "tile" t within a group is row 4p+t.
                "xb": x16[rows].reshape(128, RT * D),
                "xo": x16[opp].reshape(128, OT * D),
                "tb": tf[rows].reshape(128, RT),
                "to": np.ascontiguousarray(np.broadcast_to(to_row, (128, HALF))),
            }
        )
    return in_maps


def kernel(inputs: np.ndarray, targets: np.ndarray) -> np.ndarray:
    from concourse.bass_utils import run_bass_kernel_spmd

    nc = _get_nc()
    in_maps = make_in_maps(inputs, targets)
    res = run_bass_kernel_spmd(nc, in_maps, list(range(NCORES)))
    total = sum(float(res.results[i]["out"][0, 0]) for i in range(NCORES))
    return np.float32(total / N)


# revision 19
# speedup vs baseline: 1.9037x; 1.0101x over previous
"""Trainium2 Bass kernel for a hard-triplet margin-ranking loss.

Sharding: data-parallel over anchor rows. 8 cores x 512 rows each. Rows in
the first half of the batch mine over columns [2048:4096], rows in the second
half over [0:2048], so each core needs only its 512x2048 slice of the
distance matrix. Per core:

  1. Features arrive as fp16 (host-converted; 5e-4 relative quantization,
     far inside the 2e-2 tolerance) in five [128, 4x256] group tiles.
  2. Row norms per group: ONE wide ACT Square into fp16 scratch, then four
     4x-fast-mode DVE tensor_scalar add-accumulates (one per 256-wide tile),
     ACT sqrt on [128,4], DVE eps+reciprocal. Per-group chains let each
     group normalize as soon as its own data lands.
  3. Normalize via DVE tensor_scalar in fp16 (4x fast mode; anchors scaled
     by -0.25*inv so pm = -0.25*<xn_i,xn_j>, dist^2 = 2 + 8*pm),
     PE-transpose per group (fp16: 1 cycle/row vs 4 for fp32), evacuate
     PSUM per group with one ACT copy into fp16 xT tiles.
  4. Same-class mask built once per row tile as mq = (t_o == t_b) in fp16
     (tensor_scalar is_equal, 4x fast mode; opposite-half targets arrive
     host-pre-broadcast to [128, 2048] fp16).
  5. pm = xbT.T @ xoT on PE (fp16, fp32 PSUM, K=256) PLUS a third K-chunk
     I.T @ mq_r that adds the +1 same-class mask inside PSUM on the
     otherwise idle PE -- no separate DVE mask pass. Matched columns land
     in [0.75, 1.25], unmatched in [-0.25, 0.25].
  6. Row max (hardest positive) and row min (hardest negative), each as a
     tensor_tensor halving level straight out of PSUM (cost = output width,
     so the 2048 inputs cost 1024 cycles) followed by a 4x-fast-mode
     tensor_scalar accumulate-reduce on the fp16 [128,1024] intermediate.
  7. dist_ap = sqrt(relu(8*mx - 6)) (exact 0 when a row has no positives),
     dist_an = sqrt(relu(8*mn + 2) + eps) or 1.0 when a row has no
     negatives (mn > 0.5 only when every column is same-class),
     row loss = relu(dist_ap - dist_an + margin); ones-matmul row-sum.
  8. Host sums the 8 per-core partial sums / 4096.
"""

import numpy as np

N, D = 4096, 256
HALF = N // 2
NCORES = 8
RPC = N // NCORES  # 512 anchor rows per core
RT = RPC // 128    # 4 anchor row tiles
OT = HALF // 128   # 16 opposite-half tiles
NT = RT + OT       # 20 input tiles
NG = NT // 4       # 5 groups of 4 tiles
MARGIN = 0.3
EPS = 1e-6
S = 0.125          # anchor pre-scale: pm = -2*S*dot = -0.25*dot

_CACHE = {}


def _build():
    from contextlib import ExitStack

    import concourse.bacc as bacc
    import concourse.bass as bass
    import concourse.tile as tile
    from concourse import masks, mybir

    f32 = mybir.dt.float32
    f16 = mybir.dt.float16
    Alu = mybir.AluOpType
    Act = mybir.ActivationFunctionType
    AxX = mybir.AxisListType.X
    ts = bass.ts

    nc = bacc.Bacc(
        "TRN2",
        target_bir_lowering=False,
        debug=False,
        enable_asserts=True,
        num_devices=NCORES,
    )
    xb = nc.dram_tensor("xb", [128, RT * D], f16, kind="ExternalInput").ap()
    xo = nc.dram_tensor("xo", [128, OT * D], f16, kind="ExternalInput").ap()
    tb = nc.dram_tensor("tb", [128, RT], f32, kind="ExternalInput").ap()
    to = nc.dram_tensor("to", [128, HALF], f16, kind="ExternalInput").ap()
    out = nc.dram_tensor("out", [1, 1], f32, kind="ExternalOutput").ap()

    with tile.TileContext(nc) as tc, ExitStack() as ctx:
        const = ctx.enter_context(tc.tile_pool(name="const", bufs=1))
        xin = ctx.enter_context(tc.tile_pool(name="xin", bufs=1))
        xt = ctx.enter_context(tc.tile_pool(name="xt", bufs=1))
        stat = ctx.enter_context(tc.tile_pool(name="stat", bufs=1))
        scr = ctx.enter_context(tc.tile_pool(name="scr", bufs=3))
        wscr = ctx.enter_context(tc.tile_pool(name="wscr", bufs=2))
        mqp = ctx.enter_context(tc.tile_pool(name="mqp", bufs=4))
        tree = ctx.enter_context(tc.tile_pool(name="tree", bufs=2))
        psum = ctx.enter_context(tc.tile_pool(name="psum", bufs=2, space="PSUM"))

        ident = const.tile([128, 128], f16, tag="ident")
        masks.make_identity(nc, ident[:])
        # jneg = -I: the mask K-chunk adds -1 to same-class columns so the
        # hardest positive is the row MIN and the hardest negative the MAX.
        jneg = const.tile([128, 128], f16, tag="jneg")
        nc.gpsimd.memset(jneg[:], 0.0)
        nc.gpsimd.affine_select(
            out=jneg[:], in_=jneg[:], compare_op=Alu.not_equal,
            fill=-1.0, base=0, pattern=[[-1, 128]], channel_multiplier=1,
        )
        ones = const.tile([128, 1], f32, tag="ones")
        nc.vector.memset(ones[:], 1.0)

        # Warm both ACT function-table sets (Square/Copy and Sqrt/Relu)
        # during the DMA shadow so no LoadActFuncSet lands mid-pipeline.
        warm = const.tile([128, 1], f32, tag="warm")
        nc.scalar.activation(warm[:], ones[:], Act.Square)
        nc.scalar.activation(warm[:], ones[:], Act.Sqrt)
        nc.scalar.activation(warm[:], ones[:], Act.Relu)

        # Feature tiles in 5 groups of 4: group 0 = anchors, 1..4 =
        # opposite.  Anchors + targets on the SP DGE queue, opposite-half
        # groups on the Pool DGE queue so the first group isn't stuck
        # behind 1.5MB of queue-serial transfers.
        xg = []
        for g in range(NG):
            gt = xin.tile([128, 4 * D], f16, tag=f"xg{g}")
            if g == 0:
                nc.sync.dma_start(gt[:], xb[:])
            else:
                nc.gpsimd.dma_start(gt[:], xo[:, (g - 1) * 4 * D : g * 4 * D])
            xg.append(gt)

        # Targets: opposite-half row pre-broadcast on host to [128, HALF];
        # per-row targets as one [128, RT] per-partition scalar bank.
        tbt = const.tile([128, RT], f32, tag="tbt")
        nc.sync.dma_start(tbt[:], tb[:])
        tob = const.tile([128, HALF], f16, tag="tob")
        nc.sync.dma_start(tob[:], to[:])

        # Per group: wide square -> per-tile sum -> sqrt(4*sq) -> recip ->
        # normalize -> transpose -> evac.  Both sides scale by 0.5*inv so
        # pm = 0.25*<xn_i,xn_j> and the mask enters with -1 (jneg below).
        # Squares for the first groups are issued ahead of the chains so
        # ACT interleaves squares with evacuations without idling.
        xT = [None] * NG

        def issue_square(g):
            s2 = wscr.tile([128, 1024], f16, tag="s2")
            nc.scalar.activation(s2[:], xg[g][:], Act.Square)
            sqg = stat.tile([128, 4], f32, tag=f"sq{g}")
            for i in range(4):
                sd = scr.tile([128, D], f16, tag="sq_dummy")
                nc.vector.tensor_scalar(
                    sd[:], s2[:, ts(i, D)], 1.0, None,
                    op0=Alu.mult, op1=Alu.add, accum_out=sqg[:, i : i + 1],
                )
            return sqg

        def issue_chain(g, sqg):
            # inv = 1/(2*nrm) via sqrt(4*sq); the +eps of the reference
            # changes the result by <1e-7 and is dropped.
            nrm2 = stat.tile([128, 4], f32, tag=f"nrm{g}")
            nc.scalar.activation(nrm2[:], sqg[:], Act.Sqrt, scale=4.0)
            inv = stat.tile([128, 4], f32, tag=f"inv{g}")
            nc.vector.reciprocal(inv[:], nrm2[:])
            gt = xt.tile([128, 1024], f16, tag=f"xT{g}")
            pt = psum.tile([128, 1024], f16, tag="ps")
            for i in range(4):
                xn = scr.tile([128, D], f16, tag="xn")
                nc.vector.tensor_scalar_mul(
                    xn[:], xg[g][:, ts(i, D)], inv[:, i : i + 1]
                )
                for c in range(2):
                    nc.tensor.transpose(
                        pt[:, ts(c * 4 + i, 128)], xn[:, ts(c, 128)], ident[:]
                    )
            nc.scalar.copy(gt[:], pt[:])
            xT[g] = gt

        sqs = [issue_square(g) for g in range(3)]
        issue_chain(0, sqs[0])
        sqs.append(issue_square(3))
        issue_chain(1, sqs[1])
        sqs.append(issue_square(4))
        for g in range(2, NG):
            issue_chain(g, sqs[g])

        # Same-class masks, one per anchor row tile (4x fast mode).
        mq = []
        for r in range(RT):
            m = mqp.tile([128, HALF], f16, tag=f"mq{r}")
            nc.vector.tensor_scalar(
                m[:], tob[:], tbt[:, r : r + 1], None, op0=Alu.is_equal
            )
            mq.append(m)

        # Epilogue tiles + bias constants (epilogue runs in two halves so
        # tiles 0-1 decode while tiles 2-3 are still reducing).
        bm6 = const.tile([128, 1], f32, tag="bm6")
        nc.vector.memset(bm6[:], -6.0)
        bp2 = const.tile([128, 1], f32, tag="bp2")
        nc.vector.memset(bp2[:], 2.0)
        beps = const.tile([128, 1], f32, tag="beps")
        nc.vector.memset(beps[:], EPS)
        mx = stat.tile([128, RT], f32, tag="mx")
        mn = stat.tile([128, RT], f32, tag="mn")
        u = stat.tile([128, RT], f32, tag="u")
        dap = stat.tile([128, RT], f32, tag="dap")
        v = stat.tile([128, RT], f32, tag="v")
        sv = stat.tile([128, RT], f32, tag="sv")
        e = stat.tile([128, RT], f32, tag="e")
        ome = stat.tile([128, RT], f32, tag="ome")
        t1e = stat.tile([128, RT], f32, tag="t1e")
        dan = stat.tile([128, RT], f32, tag="dan")
        df = stat.tile([128, RT], f32, tag="df")
        lrow = stat.tile([128, RT], f32, tag="lrow")

        def epilogue(sl):
            # psum = 0.25*<xn_i,xn_j> - mask.  dist^2 = 2 - 8*psum_dot.
            # Hardest positive = row min (masked down): dist_ap^2 =
            # relu(-8*mn - 6), exact 0 when the row has no positives.
            # Hardest negative = row max: dist_an^2 = relu(-8*mx + 2) +
            # eps; mx < -0.5 only when every column is same-class.
            nc.scalar.activation(u[:, sl], mn[:, sl], Act.Relu, bias=bm6[:], scale=-8.0)
            nc.scalar.activation(dap[:, sl], u[:, sl], Act.Sqrt)
            nc.scalar.activation(v[:, sl], mx[:, sl], Act.Relu, bias=bp2[:], scale=-8.0)
            nc.scalar.activation(sv[:, sl], v[:, sl], Act.Sqrt, bias=beps[:])
            nc.vector.tensor_scalar(e[:, sl], mx[:, sl], -0.5, None, op0=Alu.is_lt)
            nc.vector.tensor_scalar(
                ome[:, sl], e[:, sl], -1.0, 1.0, op0=Alu.mult, op1=Alu.add
            )
            nc.vector.tensor_tensor(t1e[:, sl], sv[:, sl], ome[:, sl], op=Alu.mult)
            nc.vector.tensor_tensor(dan[:, sl], t1e[:, sl], e[:, sl], op=Alu.add)
            nc.vector.tensor_tensor(df[:, sl], dap[:, sl], dan[:, sl], op=Alu.subtract)
            nc.vector.tensor_scalar(
                lrow[:, sl], df[:, sl], MARGIN, 0.0, op0=Alu.add, op1=Alu.max
            )

        # Main matmul with the +1 mask folded in as a third K-chunk (lhsT =
        # identity); row max via the PSUM-evac pass, min via 4x TS-accum.
        for r in range(RT):
            pm = psum.tile([128, 2048], f32, tag="ps")
            for n in range(4):
                for c in range(2):
                    nc.tensor.matmul(
                        pm[:, ts(n, 512)],
                        lhsT=xT[0][:, c * RPC + r * 128 : c * RPC + (r + 1) * 128],
                        rhs=xT[1 + n][:, ts(c, 512)],
                        start=(c == 0),
                        stop=False,
                    )
                nc.tensor.matmul(
                    pm[:, ts(n, 512)],
                    lhsT=jneg[:],
                    rhs=mq[r][:, ts(n, 512)],
                    start=False,
                    stop=True,
                )
            # One PSUM-read pass: evac to fp16 AND row-max accumulate; then
            # the min re-reads the fp16 SBUF copy at the 4x fast mode.
            w16 = tree.tile([128, HALF], f16, tag="w16")
            nc.vector.tensor_scalar(
                w16[:], pm[:], 1.0, None,
                op0=Alu.mult, op1=Alu.max, accum_out=mx[:, r : r + 1],
            )
            dn = tree.tile([128, HALF], f16, tag="dn")
            nc.vector.tensor_scalar(
                dn[:], w16[:], 1.0, None,
                op0=Alu.mult, op1=Alu.min, accum_out=mn[:, r : r + 1],
            )
            if r == 1:
                epilogue(slice(0, 2))
        epilogue(slice(2, RT))

        # Row-sum across partitions via ones-matmul, then across row tiles.
        ps2 = psum.tile([1, RT], f32, tag="ps")
        nc.tensor.matmul(ps2[:], lhsT=ones[:], rhs=lrow[:], start=True, stop=True)
        tot = stat.tile([1, 1], f32, tag="tot")
        nc.vector.tensor_reduce(tot[:], ps2[:], axis=AxX, op=Alu.add)
        nc.sync.dma_start(out[:], tot[:])

    nc.compile()
    return nc


def _get_nc():
    if "nc" not in _CACHE:
        _CACHE["nc"] = _build()
    return _CACHE["nc"]


def make_in_maps(inputs: np.ndarray, targets: np.ndarray):
    x16 = np.ascontiguousarray(inputs, dtype=np.float32).astype(np.float16)
    tf = targets.astype(np.float32)
    t16 = targets.astype(np.float16)
    in_maps = []
    for r in range(NCORES):
        rows = slice(r * RPC, (r + 1) * RPC)
        opp = slice(HALF, N) if r * RPC < HALF else slice(0, HALF)
        # xo partition k holds rows 16k..16k+15; group n covers tile slices
        # 4n..4n+3, so distance column n*512 + i*128 + k is xo-row 16k + 4n
        # + i: permute targets to match, then pre-broadcast to all
        # partitions (replaces the on-chip GPSIMD broadcast with DMA).
        to_row = (
            t16[opp].reshape(128, 4, 4).transpose(1, 2, 0).reshape(1, HALF)
        )
        in_maps.append(
            {
                # partition p holds rows 4p..4p+3 (contiguous 2KB DMA);
                # "tile" t within a group is row 4p+t.
                "xb": x16[rows].reshape(128, RT * D),
                "xo": x16[opp].reshape(128, OT * D),
                "tb": tf[rows].reshape(128, RT),
                "to": np.ascontiguousarray(np.broadcast_to(to_row, (128, HALF))),
            }
        )
    return in_maps


def kernel(inputs: np.ndarray, targets: np.ndarray) -> np.ndarray:
    from concourse.bass_utils import run_bass_kernel_spmd

    nc = _get_nc()
    in_maps = make_in_maps(inputs, targets)
    res = run_bass_kernel_spmd(nc, in_maps, list(range(NCORES)))
    total = sum(float(res.results[i]["out"][0, 0]) for i in range(NCORES))
    return np.float32(total / N)


# revision 22
# speedup vs baseline: 1.9641x; 1.0317x over previous
"""Trainium2 Bass kernel for a hard-triplet margin-ranking loss.

Sharding: data-parallel over anchor rows. 8 cores x 512 rows each. Rows in
the first half of the batch mine over columns [2048:4096], rows in the second
half over [0:2048], so each core needs only its 512x2048 slice of the
distance matrix. Per core:

  1. Features arrive as fp16 (host-converted; 5e-4 relative quantization,
     far inside the 2e-2 tolerance) in five [128, 4x256] group tiles.
  2. Row norms per group: ONE wide ACT Square into fp16 scratch, then four
     4x-fast-mode DVE tensor_scalar add-accumulates (one per 256-wide tile),
     ACT sqrt on [128,4], DVE eps+reciprocal. Per-group chains let each
     group normalize as soon as its own data lands.
  3. Normalize via DVE tensor_scalar in fp16 (4x fast mode; anchors scaled
     by -0.25*inv so pm = -0.25*<xn_i,xn_j>, dist^2 = 2 + 8*pm),
     PE-transpose per group (fp16: 1 cycle/row vs 4 for fp32), evacuate
     PSUM per group with one ACT copy into fp16 xT tiles.
  4. Same-class mask built once per row tile as mq = (t_o == t_b) in fp16
     (tensor_scalar is_equal, 4x fast mode; opposite-half targets arrive
     host-pre-broadcast to [128, 2048] fp16).
  5. pm = xbT.T @ xoT on PE (fp16, fp32 PSUM, K=256) PLUS a third K-chunk
     I.T @ mq_r that adds the +1 same-class mask inside PSUM on the
     otherwise idle PE -- no separate DVE mask pass. Matched columns land
     in [0.75, 1.25], unmatched in [-0.25, 0.25].
  6. Row max (hardest positive) and row min (hardest negative), each as a
     tensor_tensor halving level straight out of PSUM (cost = output width,
     so the 2048 inputs cost 1024 cycles) followed by a 4x-fast-mode
     tensor_scalar accumulate-reduce on the fp16 [128,1024] intermediate.
  7. dist_ap = sqrt(relu(8*mx - 6)) (exact 0 when a row has no positives),
     dist_an = sqrt(relu(8*mn + 2) + eps) or 1.0 when a row has no
     negatives (mn > 0.5 only when every column is same-class),
     row loss = relu(dist_ap - dist_an + margin); ones-matmul row-sum.
  8. Host sums the 8 per-core partial sums / 4096.
"""

import numpy as np

N, D = 4096, 256
HALF = N // 2
NCORES = 8
RPC = N // NCORES  # 512 anchor rows per core
RT = RPC // 128    # 4 anchor row tiles
OT = HALF // 128   # 16 opposite-half tiles
NT = RT + OT       # 20 input tiles
NG = NT // 4       # 5 groups of 4 tiles
MARGIN = 0.3
EPS = 1e-6
S = 0.125          # anchor pre-scale: pm = -2*S*dot = -0.25*dot

_CACHE = {}


def _build():
    from contextlib import ExitStack

    import concourse.bacc as bacc
    import concourse.bass as bass
    import concourse.tile as tile
    from concourse import masks, mybir

    f32 = mybir.dt.float32
    f16 = mybir.dt.float16
    Alu = mybir.AluOpType
    Act = mybir.ActivationFunctionType
    AxX = mybir.AxisListType.X
    ts = bass.ts

    nc = bacc.Bacc(
        "TRN2",
        target_bir_lowering=False,
        debug=False,
        enable_asserts=True,
        num_devices=NCORES,
    )
    xb = nc.dram_tensor("xb", [128, RT * D], f16, kind="ExternalInput").ap()
    xo = nc.dram_tensor("xo", [128, OT * D], f16, kind="ExternalInput").ap()
    tb = nc.dram_tensor("tb", [128, RT], f32, kind="ExternalInput").ap()
    to = nc.dram_tensor("to", [128, HALF], f16, kind="ExternalInput").ap()
    out = nc.dram_tensor("out", [1, 1], f32, kind="ExternalOutput").ap()

    with tile.TileContext(nc) as tc, ExitStack() as ctx:
        const = ctx.enter_context(tc.tile_pool(name="const", bufs=1))
        xin = ctx.enter_context(tc.tile_pool(name="xin", bufs=1))
        xt = ctx.enter_context(tc.tile_pool(name="xt", bufs=1))
        stat = ctx.enter_context(tc.tile_pool(name="stat", bufs=1))
        scr = ctx.enter_context(tc.tile_pool(name="scr", bufs=3))
        wscr = ctx.enter_context(tc.tile_pool(name="wscr", bufs=2))
        mqp = ctx.enter_context(tc.tile_pool(name="mqp", bufs=4))
        tree = ctx.enter_context(tc.tile_pool(name="tree", bufs=2))
        psum = ctx.enter_context(tc.tile_pool(name="psum", bufs=4, space="PSUM"))

        ident = const.tile([128, 128], f16, tag="ident")
        masks.make_identity(nc, ident[:])
        # jneg = -I: the mask K-chunk adds -1 to same-class columns so the
        # hardest positive is the row MIN and the hardest negative the MAX.
        jneg = const.tile([128, 128], f16, tag="jneg")
        nc.gpsimd.memset(jneg[:], 0.0)
        nc.gpsimd.affine_select(
            out=jneg[:], in_=jneg[:], compare_op=Alu.not_equal,
            fill=-1.0, base=0, pattern=[[-1, 128]], channel_multiplier=1,
        )
        ones = const.tile([128, 1], f32, tag="ones")
        nc.vector.memset(ones[:], 1.0)

        # Warm both ACT function-table sets (Square/Copy and Sqrt/Relu)
        # during the DMA shadow so no LoadActFuncSet lands mid-pipeline.
        warm = const.tile([128, 1], f32, tag="warm")
        nc.scalar.activation(warm[:], ones[:], Act.Square)
        nc.scalar.activation(warm[:], ones[:], Act.Sqrt)
        nc.scalar.activation(warm[:], ones[:], Act.Relu)

        # Feature tiles in 5 groups of 4: group 0 = anchors, 1..4 =
        # opposite.  Anchors + targets on the SP DGE queue, opposite-half
        # groups on the Pool DGE queue so the first group isn't stuck
        # behind 1.5MB of queue-serial transfers.
        xg = []
        for g in range(NG):
            gt = xin.tile([128, 4 * D], f16, tag=f"xg{g}")
            if g == 0:
                nc.sync.dma_start(gt[:], xb[:])
            else:
                nc.gpsimd.dma_start(gt[:], xo[:, (g - 1) * 4 * D : g * 4 * D])
            xg.append(gt)

        # Targets: opposite-half row pre-broadcast on host to [128, HALF];
        # per-row targets as one [128, RT] per-partition scalar bank.
        tbt = const.tile([128, RT], f32, tag="tbt")
        nc.sync.dma_start(tbt[:], tb[:])
        tob = const.tile([128, HALF], f16, tag="tob")
        nc.sync.dma_start(tob[:], to[:])

        # Per group: wide square -> per-tile sum -> sqrt(4*sq) -> recip ->
        # normalize -> transpose -> evac.  Both sides scale by 0.5*inv so
        # pm = 0.25*<xn_i,xn_j> and the mask enters with -1 (jneg below).
        # Squares for the first groups are issued ahead of the chains so
        # ACT interleaves squares with evacuations without idling.
        xT = [None] * NG

        def issue_square(g):
            s2 = wscr.tile([128, 1024], f16, tag="s2")
            nc.scalar.activation(s2[:], xg[g][:], Act.Square)
            sqg = stat.tile([128, 4], f32, tag=f"sq{g}")
            for i in range(4):
                sd = scr.tile([128, D], f16, tag="sq_dummy")
                nc.vector.tensor_scalar(
                    sd[:], s2[:, ts(i, D)], 1.0, None,
                    op0=Alu.mult, op1=Alu.add, accum_out=sqg[:, i : i + 1],
                )
            return sqg

        def issue_chain(g, sqg):
            # inv = 1/(2*nrm) via sqrt(4*sq); the +eps of the reference
            # changes the result by <1e-7 and is dropped.
            nrm2 = stat.tile([128, 4], f32, tag=f"nrm{g}")
            nc.scalar.activation(nrm2[:], sqg[:], Act.Sqrt, scale=4.0)
            inv = stat.tile([128, 4], f32, tag=f"inv{g}")
            nc.vector.reciprocal(inv[:], nrm2[:])
            gt = xt.tile([128, 1024], f16, tag=f"xT{g}")
            pt = psum.tile([128, 1024], f16, tag="ps")
            for i in range(4):
                xn = scr.tile([128, D], f16, tag="xn")
                nc.vector.tensor_scalar_mul(
                    xn[:], xg[g][:, ts(i, D)], inv[:, i : i + 1]
                )
                for c in range(2):
                    nc.tensor.transpose(
                        pt[:, ts(c * 4 + i, 128)], xn[:, ts(c, 128)], ident[:]
                    )
            nc.scalar.copy(gt[:], pt[:])
            xT[g] = gt

        sqs = [issue_square(g) for g in range(3)]
        issue_chain(0, sqs[0])
        sqs.append(issue_square(3))
        issue_chain(1, sqs[1])
        sqs.append(issue_square(4))
        for g in range(2, NG):
            issue_chain(g, sqs[g])

        # Same-class masks, one per anchor row tile (4x fast mode).
        mq = []
        for r in range(RT):
            m = mqp.tile([128, HALF], f16, tag=f"mq{r}")
            nc.vector.tensor_scalar(
                m[:], tob[:], tbt[:, r : r + 1], None, op0=Alu.is_equal
            )
            mq.append(m)

        # Epilogue tiles + bias constants (epilogue runs in two halves so
        # tiles 0-1 decode while tiles 2-3 are still reducing).
        bm6 = const.tile([128, 1], f32, tag="bm6")
        nc.vector.memset(bm6[:], -6.0)
        bp2 = const.tile([128, 1], f32, tag="bp2")
        nc.vector.memset(bp2[:], 2.0)
        beps = const.tile([128, 1], f32, tag="beps")
        nc.vector.memset(beps[:], EPS)
        mx = stat.tile([128, RT], f32, tag="mx")
        mn = stat.tile([128, RT], f32, tag="mn")
        u = stat.tile([128, RT], f32, tag="u")
        dap = stat.tile([128, RT], f32, tag="dap")
        v = stat.tile([128, RT], f32, tag="v")
        sv = stat.tile([128, RT], f32, tag="sv")
        e = stat.tile([128, RT], f32, tag="e")
        ome = stat.tile([128, RT], f32, tag="ome")
        t1e = stat.tile([128, RT], f32, tag="t1e")
        dan = stat.tile([128, RT], f32, tag="dan")
        df = stat.tile([128, RT], f32, tag="df")
        lrow = stat.tile([128, RT], f32, tag="lrow")

        def epilogue(sl):
            # psum = 0.25*<xn_i,xn_j> - mask.  dist^2 = 2 - 8*psum_dot.
            # Hardest positive = row min (masked down): dist_ap^2 =
            # relu(-8*mn - 6), exact 0 when the row has no positives.
            # Hardest negative = row max: dist_an^2 = relu(-8*mx + 2) +
            # eps; mx < -0.5 only when every column is same-class.
            nc.scalar.activation(u[:, sl], mn[:, sl], Act.Relu, bias=bm6[:], scale=-8.0)
            nc.scalar.activation(dap[:, sl], u[:, sl], Act.Sqrt)
            nc.scalar.activation(v[:, sl], mx[:, sl], Act.Relu, bias=bp2[:], scale=-8.0)
            nc.scalar.activation(sv[:, sl], v[:, sl], Act.Sqrt, bias=beps[:])
            nc.vector.tensor_scalar(e[:, sl], mx[:, sl], -0.5, None, op0=Alu.is_lt)
            nc.vector.tensor_scalar(
                ome[:, sl], e[:, sl], -1.0, 1.0, op0=Alu.mult, op1=Alu.add
            )
            nc.vector.tensor_tensor(t1e[:, sl], sv[:, sl], ome[:, sl], op=Alu.mult)
            nc.vector.tensor_tensor(dan[:, sl], t1e[:, sl], e[:, sl], op=Alu.add)
            nc.vector.tensor_tensor(df[:, sl], dap[:, sl], dan[:, sl], op=Alu.subtract)
            nc.vector.tensor_scalar(
                lrow[:, sl], df[:, sl], MARGIN, 0.0, op0=Alu.add, op1=Alu.max
            )

        # Main matmul with the -1 mask folded in as a third K-chunk (lhsT
        # = -identity).  Each anchor row tile is processed as two
        # independent [128,1024] PSUM halves: the lo half (opposite groups
        # 1-2) reduces while groups 3-4 are still being transposed, and
        # freeing the lo PSUM buffer early lets the next row-tile pair
        # start its matmuls.  Row max/min per half via the fused
        # evac+accum tensor_scalar; tiny TT combines merge the halves.
        mxl = stat.tile([128, RT], f32, tag="mxl")
        mxh = stat.tile([128, RT], f32, tag="mxh")
        mnl = stat.tile([128, RT], f32, tag="mnl")
        mnh = stat.tile([128, RT], f32, tag="mnh")

        def mm_half(r, h):
            pmh = psum.tile([128, 1024], f32, tag="ps")
            for nn in range(2):
                n = 2 * h + nn
                for c in range(2):
                    nc.tensor.matmul(
                        pmh[:, ts(nn, 512)],
                        lhsT=xT[0][:, c * RPC + r * 128 : c * RPC + (r + 1) * 128],
                        rhs=xT[1 + n][:, ts(c, 512)],
                        start=(c == 0),
                        stop=False,
                    )
                nc.tensor.matmul(
                    pmh[:, ts(nn, 512)],
                    lhsT=jneg[:],
                    rhs=mq[r][:, ts(n, 512)],
                    start=False,
                    stop=True,
                )
            w16 = tree.tile([128, 1024], f16, tag="w16")
            nc.vector.tensor_scalar(
                w16[:], pmh[:], 1.0, None,
                op0=Alu.mult, op1=Alu.max,
                accum_out=(mxl if h == 0 else mxh)[:, r : r + 1],
            )
            dn = tree.tile([128, 1024], f16, tag="dn")
            nc.vector.tensor_scalar(
                dn[:], w16[:], 1.0, None,
                op0=Alu.mult, op1=Alu.min,
                accum_out=(mnl if h == 0 else mnh)[:, r : r + 1],
            )

        for pair in ((0, 1), (2, 3)):
            for r in pair:
                mm_half(r, 0)
            for r in pair:
                mm_half(r, 1)
            sl = slice(pair[0], pair[1] + 1)
            nc.vector.tensor_tensor(mx[:, sl], mxl[:, sl], mxh[:, sl], op=Alu.max)
            nc.vector.tensor_tensor(mn[:, sl], mnl[:, sl], mnh[:, sl], op=Alu.min)
            epilogue(sl)

        # Row-sum across partitions via ones-matmul, then across row tiles.
        ps2 = psum.tile([1, RT], f32, tag="ps")
        nc.tensor.matmul(ps2[:], lhsT=ones[:], rhs=lrow[:], start=True, stop=True)
        tot = stat.tile([1, 1], f32, tag="tot")
        nc.vector.tensor_reduce(tot[:], ps2[:], axis=AxX, op=Alu.add)
        nc.sync.dma_start(out[:], tot[:])

    nc.compile()
    return nc


def _get_nc():
    if "nc" not in _CACHE:
        _CACHE["nc"] = _build()
    return _CACHE["nc"]


def make_in_maps(inputs: np.ndarray, targets: np.ndarray):
    x16 = np.ascontiguousarray(inputs, dtype=np.float32).astype(np.float16)
    tf = targets.astype(np.float32)
    t16 = targets.astype(np.float16)
    in_maps = []
    for r in range(NCORES):
        rows = slice(r * RPC, (r + 1) * RPC)
        opp = slice(HALF, N) if r * RPC < HALF else slice(0, HALF)
        # xo partition k holds rows 16k..16k+15; group n covers tile slices
        # 4n..4n+3, so distance column n*512 + i*128 + k is xo-row 16k + 4n
        # + i: permute targets to match, then pre-broadcast to all
        # partitions (replaces the on-chip GPSIMD broadcast with DMA).
        to_row = (
            t16[opp].reshape(128, 4, 4).transpose(1, 2, 0).reshape(1, HALF)
        )
        in_maps.append(
            {
                # partition p holds rows 4p..4p+3 (contiguous 2KB DMA);
                # "tile" t within a group is row 4p+t.
                "xb": x16[rows].reshape(128, RT * D),
                "xo": x16[opp].reshape(128, OT * D),
                "tb": tf[rows].reshape(128, RT),
                "to": np.ascontiguousarray(np.broadcast_to(to_row, (128, HALF))),
            }
        )
    return in_maps


def kernel(inputs: np.ndarray, targets: np.ndarray) -> np.ndarray:
    from concourse.bass_utils import run_bass_kernel_spmd

    nc = _get_nc()
    in_maps = make_in_maps(inputs, targets)
    res = run_bass_kernel_spmd(nc, in_maps, list(range(NCORES)))
    total = sum(float(res.results[i]["out"][0, 0]) for i in range(NCORES))
    return np.float32(total / N)
